# revision 1
# baseline (speedup 1.0000x reference)
"""MiniCPM3 attention block on 8 Trainium2 NeuronCores.

Sharding: tensor-parallel over heads (5 heads/core). q_b / kv_b are
column-parallel, o_proj row-parallel (host sums the 8 partial outputs).
The low-rank a-projections + RMSNorm + RoPE preamble is replicated on
every core in transposed layout ([dim, token]) so every matmul contracts
over the partition axis. All big matmuls run in float32r (~1e-4 rel err,
full PE rate at moving-dim >= 256). The a-projection runs in two passes
over halves of the hidden dim so only half the weights are SBUF-resident;
the q latent is staged through DRAM between the preamble and q_b.
"""

import contextlib

import numpy as np

import concourse.bass as bass
import concourse.tile as tile
import concourse.mybir as mybir
from concourse import bacc
from concourse.bass_utils import run_bass_kernel_spmd
from concourse.masks import make_identity

# ---- problem constants (hardcoded per spec) ----
T = 2048
HIDDEN = 2560
N_HEADS = 40
D_NOPE = 64
D_ROPE = 32
D_QK = 96
D_V = 64
Q_RANK = 768
KV_RANK = 256
LAT = KV_RANK + D_ROPE  # 288
ROPE_THETA = 10000.0
EPS = 1e-6
SCALE = float(D_QK) ** -0.5

NCORES = 8
HPC = N_HEADS // NCORES  # 5
P = 128
D2 = D_ROPE // 2

F32 = mybir.dt.float32
F32R = mybir.dt.float32r
AF = mybir.ActivationFunctionType
ALU = mybir.AluOpType

KC_HID = HIDDEN // P  # 20
KC_H2 = KC_HID // 2   # 10 per pass
KC_Q = Q_RANK // P    # 6
KC_KV = KV_RANK // P  # 2
NT = T // P           # 16
NSL = 4               # preamble token slices
SLT = T // NSL        # 256
NCH = 4               # attention tq chunks
CHW = 512
OWK = [128, 128, 64]  # o_proj contraction chunks over 320 attn dims

LAST_RESULT = None
ts = bass.ts
ds = bass.ds


def _rope_inplace(nc, dst, tmp_pool, cos_sb, sin_sb, width=CHW):
    """NeoX-rotate rows [0:32] of dst[32, T] in place, chunked along T."""
    for c in range(T // width):
        sl = ts(c, width)
        t1 = tmp_pool.tile([D2, width], F32, tag="rope_t1")
        t2 = tmp_pool.tile([D2, width], F32, tag="rope_t2")
        t3 = tmp_pool.tile([D2, width], F32, tag="rope_t3")
        t4 = tmp_pool.tile([D2, width], F32, tag="rope_t4")
        nc.vector.tensor_mul(t1[:], dst[0:D2, sl], cos_sb[:, sl])
        nc.vector.tensor_mul(t2[:], dst[D2:D_ROPE, sl], sin_sb[:, sl])
        nc.vector.tensor_mul(t3[:], dst[D2:D_ROPE, sl], cos_sb[:, sl])
        nc.vector.tensor_mul(t4[:], dst[0:D2, sl], sin_sb[:, sl])
        nc.vector.tensor_sub(dst[0:D2, sl], t1[:], t2[:])
        nc.vector.tensor_add(dst[D2:D_ROPE, sl], t3[:], t4[:])


def _ropeX(nc, pool, x, csd, snd, b):
    """NeoX-rotate an AP x=[32, T] living at partitions b:b+32 in place.
    csd/snd rows b:b+32 hold [cos; cos] / [-sin; +sin].
    x_new = x * cs + swap_halves(x) * snsign."""
    bsw = pool.tile([D_QK, T], F32, tag="rope_bsw")
    nc.sync.dma_start(bsw[b:b + 16, :], x[16:32, :].bitcast(F32))
    nc.sync.dma_start(bsw[b + 16:b + 32, :], x[0:16, :].bitcast(F32))
    nc.vector.tensor_mul(bsw[b:b + 32, :], bsw[b:b + 32, :], snd[b:b + 32, :])
    nc.vector.tensor_mul(x, x, csd[b:b + 32, :])
    nc.vector.tensor_add(x, x, bsw[b:b + 32, :])


def _body(nc, tc, d, dbg=False, phase=4):
    small = tc.alloc_tile_pool(name="small", bufs=1)
    outp = tc.alloc_tile_pool(name="outp", bufs=2)
    dram = tc.alloc_tile_pool(name="dram", bufs=1, space="DRAM")
    latkv_p = tc.alloc_tile_pool(name="latkv", bufs=1, side="right")
    latq_p = tc.alloc_tile_pool(name="latq", bufs=1, side="right")

    # ---- constants ----
    ident = small.tile([P, P], F32)
    make_identity(nc, ident[:])
    # mask[i, j] = 1 if i <= j else 0 (valid: tk row <= tq col in diag blk)
    mask = small.tile([P, P], F32)
    nc.gpsimd.memset(mask[:], 1.0)
    nc.gpsimd.affine_select(
        out=mask[:], in_=mask[:], compare_op=ALU.is_ge,
        fill=0.0, base=0, pattern=[[1, P]], channel_multiplier=-1,
    )
    ones_c = small.tile([P, 1], F32)
    nc.gpsimd.memset(ones_c[:], 1.0)
    ones_r = small.tile([P, 1], F32R)
    nc.vector.tensor_copy(ones_r[:], ones_c[:])
    eps_c = small.tile([1, 1], F32)
    nc.gpsimd.memset(eps_c[:], EPS)
    sums_q = small.tile([1, T], F32)
    sums_kv = small.tile([1, T], F32)

    lat_kv = latkv_p.tile([P, KC_KV, T], F32R)
    lat_pe = latkv_p.tile([D_ROPE, T], F32R)
    lat_q = latq_p.tile([P, KC_Q, T], F32R)
    latq_dram = dram.tile([P, KC_Q, T], F32R)

    # m-chunks of the fused a-projection output (q rank | kv rank | k_pe)
    mlist = [("q", i, P) for i in range(KC_Q)]
    mlist += [("kv", i, P) for i in range(KC_KV)]
    mlist += [("pe", 0, D_ROPE)]

    # ============ preamble: lat = a_proj(hidden^T), 2 K-passes ============
    wpool = tc.alloc_tile_pool(name="wpool", bufs=1)
    wstage = tc.alloc_tile_pool(name="wstage", bufs=2)
    hraw = tc.alloc_tile_pool(name="hraw", bufs=2)
    hidtp = tc.alloc_tile_pool(name="hidtp", bufs=1)
    ps_t = tc.alloc_tile_pool(name="ps_t", bufs=2, space="PSUM")
    ps_a = tc.alloc_tile_pool(name="ps_a", bufs=3, space="PSUM")

    qaw_r = d["qaw"].rearrange("(o p) m -> p o m", p=P)
    kvaw_r = d["kvaw"].rearrange("(o p) m -> p o m", p=P)
    for half in range(2):
        qaw = wpool.tile([P, KC_H2, Q_RANK], F32R, tag="qaw")
        kvaw = wpool.tile([P, KC_H2, LAT], F32R, tag="kvaw")
        for k in range(KC_H2):
            st = wstage.tile([P, Q_RANK + LAT], F32, tag="wst")
            nc.sync.dma_start(st[:, :Q_RANK], qaw_r[:, half * KC_H2 + k])
            nc.sync.dma_start(st[:, Q_RANK:], kvaw_r[:, half * KC_H2 + k])
            nc.vector.tensor_copy(qaw[:, k], st[:, :Q_RANK])
            nc.vector.tensor_copy(kvaw[:, k], st[:, Q_RANK:])

        for s in range(NSL):
            hidT = hidtp.tile([P, KC_H2, SLT], F32R, tag="hidT")
            for tt in range(SLT // P):
                hr = hraw.tile([P, HIDDEN // 2], F32, tag="hr")
                nc.sync.dma_start(
                    hr[:],
                    d["hid"][ts(s * (SLT // P) + tt, P),
                             ds(half * (HIDDEN // 2), HIDDEN // 2)],
                )
                for k in range(KC_H2):
                    pt = ps_t.tile([P, P], F32, tag="pt")
                    nc.tensor.transpose(pt[:], hr[:, ts(k, P)], ident[:])
                    nc.vector.tensor_copy(hidT[:, k, ts(tt, P)], pt[:])

            for kind, mi, msz in mlist:
                pm = ps_a.tile([P, SLT], F32, tag="pm")
                if kind == "q":
                    w_ap = qaw[:, :, ts(mi, P)]
                    dst = lat_q[:, mi, ts(s, SLT)]
                elif kind == "kv":
                    w_ap = kvaw[:, :, ts(mi, P)]
                    dst = lat_kv[:, mi, ts(s, SLT)]
                else:
                    w_ap = kvaw[:, :, KV_RANK:LAT]
                    dst = lat_pe[:, ts(s, SLT)]
                pm_out = pm[:msz]
                for k in range(KC_H2):
                    nc.tensor.matmul(
                        pm_out, w_ap[:, k], hidT[:, k],
                        start=(k == 0), stop=(k == KC_H2 - 1),
                    )
                if half == 0:
                    nc.vector.tensor_copy(dst, pm_out)
                else:
                    nc.vector.tensor_add(dst, pm_out, dst)
    ps_a.release()
    ps_t.release()
    hidtp.release()
    hraw.release()
    wstage.release()
    wpool.release()

    # per-head weights pool (lives until o_proj starts)
    b1w = tc.alloc_tile_pool(name="b1w", bufs=1)
    qb = b1w.tile([P, KC_Q, HPC * D_QK], F32R)
    kvb = b1w.tile([P, KC_KV, HPC * (D_NOPE + D_V)], F32R)
    lnq = b1w.tile([P, KC_Q], F32)
    lnkv = b1w.tile([P, KC_KV], F32)
    cs_pool = tc.alloc_tile_pool(name="cs_pool", bufs=1)
    csd = cs_pool.tile([D_QK, T], F32)
    nc.sync.dma_start(csd[0:32, :], d["cos2T"])
    nc.sync.dma_start(csd[64:96, :], d["cos2T"])
    snd = cs_pool.tile([D_QK, T], F32)
    nc.sync.dma_start(snd[0:32, :], d["sin2T"])
    nc.sync.dma_start(snd[64:96, :], d["sin2T"])

    # ---- b0: rmsnorm + rope(k_pe) + round per-head weights ----
    b0p = tc.alloc_tile_pool(name="b0p", bufs=1)
    x2p = tc.alloc_tile_pool(name="x2p", bufs=3)
    ps_s = tc.alloc_tile_pool(name="ps_s", bufs=2, space="PSUM")


    for name_, lt, kc, srow in (
        ("q", lat_q, KC_Q, sums_q), ("kv", lat_kv, KC_KV, sums_kv),
    ):
        for c in range(NCH):
            sp = ps_s.tile([1, CHW], F32, tag="sq_" + name_)
            for k in range(kc):
                x2 = x2p.tile([P, CHW], F32R, tag="x2")
                nc.scalar.activation(x2[:], lt[:, k, ts(c, CHW)], AF.Square)
                nc.tensor.matmul(
                    sp[:], ones_r[:], x2[:],
                    start=(k == 0), stop=(k == kc - 1),
                )
            nc.vector.tensor_copy(srow[:, ts(c, CHW)], sp[:])

    nc.scalar.activation(sums_q[:], sums_q[:], AF.Sqrt,
                         bias=eps_c[:], scale=1.0 / Q_RANK)
    nc.vector.reciprocal(sums_q[:], sums_q[:])
    nc.scalar.activation(sums_kv[:], sums_kv[:], AF.Sqrt,
                         bias=eps_c[:], scale=1.0 / KV_RANK)
    nc.vector.reciprocal(sums_kv[:], sums_kv[:])
    for name_, lt, kc, srow in (
        ("q", lat_q, KC_Q, sums_q), ("kv", lat_kv, KC_KV, sums_kv),
    ):
        bcast = b0p.tile([P, T], F32, tag="bcast")
        nc.gpsimd.partition_broadcast(bcast[:], srow[:])
        for k in range(kc):
            nc.vector.tensor_mul(lt[:, k], lt[:, k], bcast[:])

    _ropeX(nc, b0p, lat_pe[:, :], csd, snd, 0)

    if dbg:
        nc.sync.dma_start(d["dbg_latq"], lat_q[:].bitcast(F32))
        nc.sync.dma_start(d["dbg_latkv"], lat_kv[:].bitcast(F32))
        nc.sync.dma_start(d["dbg_latpe"], lat_pe[:].bitcast(F32))
        nc.sync.dma_start(d["dbg_sumsq"], sums_q[:])
        nc.sync.dma_start(d["dbg_sumskv"], sums_kv[:])

    # fold q_a_ln * SCALE into q_b, kv_a_ln into kv_b
    nc.sync.dma_start(lnq[:], d["qln"].rearrange("(o p) -> p o", p=P))
    nc.sync.dma_start(lnkv[:], d["kvln"].rearrange("(o p) -> p o", p=P))
    qb_st = b0p.tile([P, KC_Q, HPC * D_QK], F32, tag="wst2")
    nc.sync.dma_start(qb_st[:], d["qb"].rearrange("(o p) m -> p o m", p=P))
    nc.vector.scalar_tensor_tensor(
        qb[:], qb_st[:], SCALE,
        lnq[:, :, None].to_broadcast([P, KC_Q, HPC * D_QK]),
        op0=ALU.mult, op1=ALU.mult,
    )
    kvb_st = b0p.tile([P, KC_KV, HPC * (D_NOPE + D_V)], F32, tag="wst2")
    nc.sync.dma_start(kvb_st[:], d["kvb"].rearrange("(o p) m -> p o m", p=P))
    nc.vector.tensor_mul(
        kvb[:], kvb_st[:],
        lnkv[:, :, None].to_broadcast([P, KC_KV, HPC * (D_NOPE + D_V)]),
    )

    # stage lat_q out to DRAM so its SBUF can be reused
    nc.sync.dma_start(latq_dram[:], lat_q[:])
    ps_s.release()
    x2p.release()
    b0p.release()
    latq_p.release()

    if phase < 2:
        nc.sync.dma_start(d["out"][0:P, 0:1], ones_c[:])
        cs_pool.release()
        latkv_p.release()
        b1w.release()
        outp.release()
        dram.release()
        small.release()
        return

    # ---- big phase-B tensors ----
    qkv = tc.alloc_tile_pool(name="qkv", bufs=1, side="right")
    qT = qkv.tile([D_QK, HPC, T], F32R)
    kT = qkv.tile([D_QK, HPC, T], F32R)
    V = qkv.tile([P, NT, HPC, D_V + 1], F32R)

    # ---- b1b: kT / V from lat_kv ----
    ps_k = tc.alloc_tile_pool(name="ps_k", bufs=2, space="PSUM")
    ps_v = tc.alloc_tile_pool(name="ps_v", bufs=2, space="PSUM")
    for h in range(HPC):
        for c in range(NCH):
            kps = ps_k.tile([P, CHW], F32, tag="kps")
            for k in range(KC_KV):
                nc.tensor.matmul(
                    kps[:D_NOPE], kvb[:, k, ds(h * P, D_NOPE)],
                    lat_kv[:, k, ts(c, CHW)],
                    start=(k == 0), stop=(k == KC_KV - 1),
                )
            nc.vector.tensor_copy(kT[:D_NOPE, h, ts(c, CHW)], kps[:D_NOPE])
        nc.sync.dma_start(kT[D_NOPE:D_QK, h], lat_pe[:, :])
        nc.vector.tensor_copy(kT[D_NOPE:D_QK, h], kT[D_NOPE:D_QK, h].bitcast(F32))

    vcols = kvb.rearrange("p k (h d) -> p k h d", h=HPC)[:, :, :, D_NOPE:]
    for tt in range(NT):
        vps = ps_v.tile([P, HPC * D_V], F32, tag="vps")
        for k in range(KC_KV):
            nc.tensor.matmul(
                vps[:], lat_kv[:, k, ts(tt, P)], vcols[:, k],
                start=(k == 0), stop=(k == KC_KV - 1),
            )
        nc.vector.tensor_copy(
            V[:, tt, :, 0:D_V],
            vps[:].rearrange("p (h d) -> p h d", h=HPC),
        )
    nc.vector.tensor_copy(
        V[:, :, :, D_V:D_V + 1],
        ones_c[:, :, None, None].to_broadcast([P, NT, HPC, 1]),
    )
    ps_v.release()
    ps_k.release()

    # ---- b1a: qT = qb^T @ lat_q (streamed back from DRAM) + rope(q_pe) ----
    lqst_p = tc.alloc_tile_pool(name="lqst", bufs=1)
    lqc_p = tc.alloc_tile_pool(name="lqc", bufs=1)
    ps_q = tc.alloc_tile_pool(name="ps_q", bufs=1, space="PSUM")
    for c in range(NCH):
        lq_st = lqst_p.tile([P, KC_Q, CHW], F32, tag="lqst")
        nc.sync.dma_start(lq_st[:], latq_dram[:, :, ts(c, CHW)].bitcast(F32))
        lqc = lqc_p.tile([P, KC_Q, CHW], F32R, tag="lqc")
        nc.vector.tensor_copy(lqc[:], lq_st[:])
        qps = [ps_q.tile([P, CHW], F32, tag="qps%d" % h, name="qps%d" % h) for h in range(HPC)]
        for k in range(KC_Q):
            for h in range(HPC):
                nc.tensor.matmul(
                    qps[h][:D_QK], qb[:, k, ts(h, D_QK)], lqc[:, k],
                    start=(k == 0), stop=(k == KC_Q - 1),
                )
        for h in range(HPC):
            nc.vector.tensor_copy(qT[:, h, ts(c, CHW)], qps[h][:D_QK])
    ps_q.release()
    lqc_p.release()
    lqst_p.release()
    rtmp2 = tc.alloc_tile_pool(name="rtmp2", bufs=2)
    for h in range(HPC):
        _ropeX(nc, rtmp2, qT[D_NOPE:D_QK, h], csd, snd, 64)
    rtmp2.release()
    cs_pool.release()

    if dbg:
        nc.sync.dma_start(d["dbg_qT"], qT[:].bitcast(F32))
        nc.sync.dma_start(d["dbg_kT"], kT[:].bitcast(F32))
        nc.sync.dma_start(d["dbg_V"], V[:].bitcast(F32))

    if phase < 3:
        nc.sync.dma_start(d["out"][0:P, 0:1], ones_c[:])
        nc.sync.dma_start(d["out"][0:P, 1:2], qT[0:P - 32, 0, 0:1].bitcast(F32))
        qkv.release()
        latkv_p.release()
        b1w.release()
        outp.release()
        dram.release()
        small.release()
        return

    # ================= attention =================
    attnp = tc.alloc_tile_pool(name="attnp", bufs=1)
    attnT = attnp.tile([P, 3, T], F32R)

    exp_p = tc.alloc_tile_pool(name="exp_p", bufs=4)
    nrm_p = tc.alloc_tile_pool(name="nrm_p", bufs=1)
    ps_sc = tc.alloc_tile_pool(name="ps_sc", bufs=3, space="PSUM")
    ps_pv = tc.alloc_tile_pool(name="ps_pv", bufs=3, space="PSUM")

    for h in range(HPC):
        for c in range(NCH):
            pv = ps_pv.tile([P, CHW], F32, tag="pv")
            njt = 4 * c + 4
            for j in range(njt):
                d0 = max(0, P * (j - 4 * c))
                w = CHW - d0
                sps = ps_sc.tile([P, CHW], F32, tag="sps")
                nc.tensor.matmul(
                    sps[:, d0:], kT[:, h, ts(j, P)],
                    qT[:, h, ds(c * CHW + d0, w)],
                    start=True, stop=True,
                )
                ex = exp_p.tile([P, CHW], F32R, tag="ex")
                nc.scalar.activation(ex[:, d0:], sps[:, d0:], AF.Exp)
                if j >= 4 * c:
                    nc.vector.tensor_mul(
                        ex[:, d0:d0 + P], ex[:, d0:d0 + P], mask[:]
                    )
                nc.tensor.matmul(
                    pv[:D_V + 1, d0:], V[:, j, h], ex[:, d0:],
                    start=(j == 0), stop=(j == njt - 1),
                )
            rs = nrm_p.tile([D_V + 1, CHW], F32, tag="rs")
            nc.vector.tensor_copy(rs[D_V:D_V + 1, :], pv[D_V:D_V + 1, :])
            rc = nrm_p.tile([1, CHW], F32, tag="rc")
            nc.sync.dma_start(rc[:], rs[D_V:D_V + 1, :])
            nc.vector.reciprocal(rc[:], rc[:])
            bc = nrm_p.tile([D_V, CHW], F32, tag="bc")
            nc.gpsimd.partition_broadcast(bc[:], rc[:])
            atmp = nrm_p.tile([D_V, CHW], F32R, tag="atmp")
            nc.vector.tensor_mul(atmp[:], pv[0:D_V, :], bc[:])
            nc.sync.dma_start(
                attnT[(h % 2) * D_V:(h % 2) * D_V + D_V, h // 2, ts(c, CHW)],
                atmp[:],
            )
    for kc in range(3):
        nc.vector.tensor_copy(attnT[:, kc], attnT[:, kc].bitcast(F32))
    if dbg:
        nc.sync.dma_start(d["dbg_attnT"], attnT[:].bitcast(F32))
    ps_pv.release()
    ps_sc.release()
    nrm_p.release()
    exp_p.release()
    qkv.release()
    latkv_p.release()

    if phase < 4:
        nc.sync.dma_start(d["out"][0:P, 0:1], ones_c[:])
        nc.sync.dma_start(d["out"][0:P, 2:3], attnT[:, 0, 0:1].bitcast(F32))
        attnp.release()
        b1w.release()
        outp.release()
        dram.release()
        small.release()
        return

    # ================= o_proj (row-parallel partial) =================
    owp = tc.alloc_tile_pool(name="owp", bufs=1)
    ow_st_p = tc.alloc_tile_pool(name="ow_st", bufs=2)
    ps_o = tc.alloc_tile_pool(name="ps_o", bufs=3, space="PSUM")
    ow = owp.tile([P, 3, HIDDEN], F32R)
    for kc in range(3):
        rows = OWK[kc]
        owst = ow_st_p.tile([P, HIDDEN], F32, tag="owst")
        nc.sync.dma_start(owst[:rows], d["ow"][ds(kc * P, rows), :])
        nc.vector.tensor_copy(ow[:rows, kc], owst[:rows])
    for t in range(NT):
        for grp in ([0, 1], [2, 3], [4]):
            gw = len(grp) * CHW
            g0 = grp[0] * CHW
            ob = outp.tile([P, 2 * CHW], F32, tag="ob")
            for n in grp:
                ops = ps_o.tile([P, CHW], F32, tag="ops")
                for kc in range(3):
                    rows = OWK[kc]
                    nc.tensor.matmul(
                        ops[:], attnT[:rows, kc, ts(t, P)],
                        ow[:rows, kc, ts(n, CHW)],
                        start=(kc == 0), stop=(kc == 2),
                    )
                if (t + n) % 2 == 0:
                    nc.vector.tensor_copy(ob[:, ds(n * CHW - g0, CHW)], ops[:])
                else:
                    nc.scalar.copy(ob[:, ds(n * CHW - g0, CHW)], ops[:])
            nc.sync.dma_start(d["out"][ts(t, P), ds(g0, gw)], ob[:, :gw])
    ps_o.release()
    ow_st_p.release()
    owp.release()
    attnp.release()
    b1w.release()
    outp.release()
    dram.release()
    small.release()


def _build(dbg=False, phase=4):
    nc = bacc.Bacc("TRN2", target_bir_lowering=False, debug=False, num_devices=NCORES)
    d = {
        "hid": nc.dram_tensor("hid", [T, HIDDEN], F32, kind="ExternalInput").ap(),
        "qaw": nc.dram_tensor("qaw", [HIDDEN, Q_RANK], F32, kind="ExternalInput").ap(),
        "kvaw": nc.dram_tensor("kvaw", [HIDDEN, LAT], F32, kind="ExternalInput").ap(),
        "qb": nc.dram_tensor("qb", [Q_RANK, HPC * D_QK], F32, kind="ExternalInput").ap(),
        "kvb": nc.dram_tensor("kvb", [KV_RANK, HPC * (D_NOPE + D_V)], F32, kind="ExternalInput").ap(),
        "ow": nc.dram_tensor("ow", [HPC * D_V, HIDDEN], F32, kind="ExternalInput").ap(),
        "qln": nc.dram_tensor("qln", [Q_RANK], F32, kind="ExternalInput").ap(),
        "kvln": nc.dram_tensor("kvln", [KV_RANK], F32, kind="ExternalInput").ap(),
        "cos2T": nc.dram_tensor("cos2T", [D_ROPE, T], F32, kind="ExternalInput").ap(),
        "sin2T": nc.dram_tensor("sin2T", [D_ROPE, T], F32, kind="ExternalInput").ap(),
        "out": nc.dram_tensor("out", [T, HIDDEN], F32, kind="ExternalOutput").ap(),
    }
    if dbg:
        for nm, shp in (
            ("dbg_latq", [P, KC_Q, T]), ("dbg_latkv", [P, KC_KV, T]),
            ("dbg_latpe", [D_ROPE, T]), ("dbg_sumsq", [1, T]),
            ("dbg_sumskv", [1, T]), ("dbg_qT", [D_QK, HPC, T]),
            ("dbg_kT", [D_QK, HPC, T]), ("dbg_V", [P, NT, HPC, D_V + 1]),
            ("dbg_attnT", [P, 3, T]),
        ):
            d[nm] = nc.dram_tensor(nm, shp, F32, kind="ExternalOutput").ap()
    with tile.TileContext(nc) as tc:
        _body(nc, tc, d, dbg=dbg, phase=phase)
    nc.compile()
    return nc


def make_in_maps(positions, hidden_states, q_a_w, q_a_ln, q_b_w, kv_a_w,
                 kv_a_ln, kv_b_w, o_w):
    pos = np.asarray(positions)
    hid = np.ascontiguousarray(np.asarray(hidden_states, dtype=np.float32))

    inv_freq = 1.0 / (ROPE_THETA ** (np.arange(0, D_ROPE, 2, dtype=np.float32) / D_ROPE))
    freqs = pos.astype(np.float32)[:, None] * inv_freq[None, :]  # [T, 16]
    cosv = np.cos(freqs).T.astype(np.float32)  # [16, T]
    sinv = np.sin(freqs).T.astype(np.float32)
    cos2T = np.ascontiguousarray(np.concatenate([cosv, cosv], axis=0))
    sin2T = np.ascontiguousarray(np.concatenate([-sinv, sinv], axis=0))

    q_b_w = np.asarray(q_b_w, dtype=np.float32)
    kv_b_w = np.asarray(kv_b_w, dtype=np.float32)
    o_w = np.asarray(o_w, dtype=np.float32)

    in_maps = []
    for c in range(NCORES):
        in_maps.append({
            "hid": hid,
            "qaw": np.ascontiguousarray(np.asarray(q_a_w, dtype=np.float32)),
            "kvaw": np.ascontiguousarray(np.asarray(kv_a_w, dtype=np.float32)),
            "qb": np.ascontiguousarray(q_b_w[:, c * HPC * D_QK:(c + 1) * HPC * D_QK]),
            "kvb": np.ascontiguousarray(
                kv_b_w[:, c * HPC * (D_NOPE + D_V):(c + 1) * HPC * (D_NOPE + D_V)]),
            "ow": np.ascontiguousarray(o_w[c * HPC * D_V:(c + 1) * HPC * D_V, :]),
            "qln": np.ascontiguousarray(np.asarray(q_a_ln, dtype=np.float32)),
            "kvln": np.ascontiguousarray(np.asarray(kv_a_ln, dtype=np.float32)),
            "cos2T": cos2T,
            "sin2T": sin2T,
        })
    return in_maps


def kernel(positions, hidden_states, q_a_w, q_a_ln, q_b_w, kv_a_w, kv_a_ln,
           kv_b_w, o_w, trace=False):
    global LAST_RESULT
    in_maps = make_in_maps(positions, hidden_states, q_a_w, q_a_ln, q_b_w,
                           kv_a_w, kv_a_ln, kv_b_w, o_w)
    nc = _build()
    res = run_bass_kernel_spmd(nc, in_maps, core_ids=list(range(NCORES)), trace=trace)
    LAST_RESULT = res
    parts = np.stack([res.results[c]["out"] for c in range(NCORES)], axis=0)
    return parts.sum(axis=0, dtype=np.float64).astype(np.float32)



# revision 9
# speedup vs baseline: 6.9888x; 6.9888x over previous
"""MiniCPM3 attention block on 8 Trainium2 NeuronCores — v2.

Sharding: tensor-parallel over heads (5 heads/core); q_b/kv_b column-parallel,
o_proj row-parallel (host sums the 8 partial outputs). The low-rank
a-projection preamble is replicated per core (collectives have a ~78ms
per-call fixed cost in this environment and are avoided).

Key tricks vs v1:
- bf16 everywhere on-chip (weights pre-cast host-side); fp32 PSUM accums.
- RoPE via weights: the host appends "swapped" columns (sw = [-x2; x1]) to
  the pe slices of kv_a_w and q_b_w. Roping then becomes an elementwise
  multiply by [cos;cos]/[sin;sin] rows (fused into the PSUM evacuation for
  q) plus, for k only, a cross-partition add done with two identity
  matmuls. The q-side add happens implicitly inside the scores matmul by
  duplicating the roped k rows (contraction over 128 rows).
- ln and SCALE folded into q_b/kv_b host-side.
- attention chunks of 1024 columns (one exp per (h,j)), o_proj interleaved
  between attention heads so PE fills ACT-bound gaps.
- bf16 output, cast+summed on host.
"""

import numpy as np

import concourse.bass as bass
import concourse.tile as tile
import concourse.mybir as mybir
from concourse import bacc
from concourse.bass_utils import run_bass_kernel_spmd
from concourse.masks import make_identity

# ---- problem constants (hardcoded per spec) ----
T = 2048
HIDDEN = 2560
N_HEADS = 40
D_NOPE = 64
D_ROPE = 32
D_QK = 96
D_V = 64
Q_RANK = 768
KV_RANK = 256
ROPE_THETA = 10000.0
EPS = 1e-6
SCALE = float(D_QK) ** -0.5

NCORES = 8
HPC = N_HEADS // NCORES  # 5
P = 128
D2 = D_ROPE // 2  # 16

F32 = mybir.dt.float32
F32R = mybir.dt.float32r
BF16 = mybir.dt.bfloat16
AF = mybir.ActivationFunctionType
ALU = mybir.AluOpType

KC_HID = HIDDEN // P          # 20
KC_Q = Q_RANK // P            # 6
KC_KV = KV_RANK // P          # 2
MTOT = Q_RANK + KV_RANK + 2 * D_ROPE  # 1088 (q | kv | pe | sw)
NT = T // P                   # 16
NSL = 4                       # preamble token slices
SLT = T // NSL                # 512
CHW = 1024                    # attention tq chunk width
NCH = T // CHW                # 2
JPC = CHW // P                # 8 k-blocks per chunk

LAST_RESULT = None
ts = bass.ts
ds = bass.ds


def _body(nc, tc, d, dbg=False, phase=4):
    small = tc.alloc_tile_pool(name="small", bufs=1)
    ident = small.tile([P, P], F32)
    make_identity(nc, ident[:])
    # mask[i, j] = 1 if i <= j else 0 (tk row i <= tq col j in diag block)
    maskf = small.tile([P, P], F32)
    nc.gpsimd.memset(maskf[:], 1.0)
    nc.gpsimd.affine_select(
        out=maskf[:], in_=maskf[:], compare_op=ALU.is_ge,
        fill=0.0, base=0, pattern=[[1, P]], channel_multiplier=-1,
    )
    mask = small.tile([P, P], BF16)
    nc.vector.tensor_copy(mask[:], maskf[:])
    ones_b = small.tile([P, 1], BF16)
    nc.gpsimd.memset(ones_b[:], 1.0)
    eps_c = small.tile([1, 1], F32)
    nc.gpsimd.memset(eps_c[:], EPS)

    # rope multiplier rows: 0:64 ones, 64:96 [c;c], 96:128 [s;s]
    rmf = small.tile([P, T], F32)

    # per-head weights (live until o_proj); tiles allocated now, DMAs
    # emitted at the end of the preamble so aw wins the sync queue
    b1w = tc.alloc_tile_pool(name="b1w", bufs=1, side="right")
    qb = b1w.tile([P, KC_Q, HPC * P], BF16)
    kvb = b1w.tile([P, KC_KV, HPC * P], BF16)
    ow = b1w.tile([P, 3, HIDDEN], BF16)

    latp = tc.alloc_tile_pool(name="latp", bufs=1)
    lat_q = latp.tile([P, KC_Q, T], BF16)
    lat_kv = latp.tile([P, KC_KV, T], BF16)
    lat_pe = latp.tile([P, T], BF16)  # rows 64:128: pe*cos / sw*sin

    # ============ preamble: lat = a_proj(hidden^T) ============
    awp = tc.alloc_tile_pool(name="awp", bufs=1)
    aw = awp.tile([P, KC_HID, MTOT], BF16)
    nc.sync.dma_start(aw[:], d["aw"].rearrange("(k p) m -> p k m", p=P))
    nc.sync.dma_start(rmf[:], d["ropemul"])

    hraw = tc.alloc_tile_pool(name="hraw", bufs=2)
    hidtp = tc.alloc_tile_pool(name="hidtp", bufs=2)
    ps_t = tc.alloc_tile_pool(name="ps_t", bufs=3, space="PSUM")
    ps_a = tc.alloc_tile_pool(name="ps_a", bufs=3, space="PSUM")
    ps_s = tc.alloc_tile_pool(name="ps_s", bufs=1, space="PSUM")
    x2p = tc.alloc_tile_pool(name="x2p", bufs=3)
    nrm0 = tc.alloc_tile_pool(name="nrm0", bufs=2)

    # m-chunks: 6 q + 2 kv + 1 pe(64 rows at partition 64)
    mlist = [("q", i) for i in range(KC_Q)] + [("kv", i) for i in range(KC_KV)]
    mlist.append(("pe", 0))

    def emit_transposes(s):
        hidT = hidtp.tile([P, KC_HID, SLT], BF16, tag="hidT",
                          name="hidT%d" % s)
        for tt in range(SLT // P):
            hr = hraw.tile([P, HIDDEN], F32, tag="hr")
            nc.scalar.dma_start(hr[:], d["hid"][ts(s * (SLT // P) + tt, P), :])
            for k4 in range(KC_HID // 4):
                pt = ps_t.tile([P, 4, P], F32, tag="pt")
                for dk in range(4):
                    nc.tensor.transpose(pt[:, dk], hr[:, ts(4 * k4 + dk, P)],
                                        ident[:])
                nc.vector.tensor_copy(
                    hidT[:, 4 * k4:4 * k4 + 4, ts(tt, P)], pt[:])
        return hidT

    hidT_next = emit_transposes(0)
    for s in range(NSL):
        sl = ts(s, SLT)
        hidT = hidT_next
        if s + 1 < NSL:
            hidT_next = emit_transposes(s + 1)
        sqq = ps_s.tile([1, SLT], F32, tag="sqq")
        sqkv = ps_s.tile([1, SLT], F32, tag="sqkv")
        for kind, mi in mlist:
            pm = ps_a.tile([P, SLT], F32, tag="pm")
            if kind == "q":
                w_ap = aw[:, :, ts(mi, P)]
                pmv = pm[:]
            elif kind == "kv":
                w_ap = aw[:, :, ds(Q_RANK + mi * P, P)]
                pmv = pm[:]
            else:
                w_ap = aw[:, :, ds(Q_RANK + KV_RANK, 2 * D_ROPE)]
                pmv = pm[D_NOPE:P]
            for k in range(KC_HID):
                nc.tensor.matmul(
                    pmv, w_ap[:, k], hidT[:, k],
                    start=(k == 0), stop=(k == KC_HID - 1),
                )
            if kind == "pe":
                # fuse the rope multiplier into evacuation (no norm on pe)
                nc.vector.tensor_mul(
                    lat_pe[D_NOPE:P, sl], pmv, rmf[D_NOPE:P, sl])
            else:
                x2 = x2p.tile([P, SLT], BF16, tag="x2")
                nc.scalar.activation(x2[:], pm[:], AF.Square)
                sq = sqq if kind == "q" else sqkv
                kc = KC_Q if kind == "q" else KC_KV
                nc.tensor.matmul(
                    sq[:], ones_b[:], x2[:],
                    start=(mi == 0), stop=(mi == kc - 1),
                )
                dst = lat_q[:, mi, sl] if kind == "q" else lat_kv[:, mi, sl]
                nc.vector.tensor_copy(dst, pm[:])
        # rmsnorm scale for this token slice
        for name_, sq, rank, lt, kc in (
            ("q", sqq, Q_RANK, lat_q, KC_Q),
            ("kv", sqkv, KV_RANK, lat_kv, KC_KV),
        ):
            rr = nrm0.tile([1, SLT], F32, tag="rr")
            nc.scalar.activation(rr[:], sq[:], AF.Sqrt,
                                 bias=eps_c[:], scale=1.0 / rank)
            nc.vector.reciprocal(rr[:], rr[:])
            rrb = nrm0.tile([1, SLT], BF16, tag="rrb")
            nc.vector.tensor_copy(rrb[:], rr[:])
            bq = nrm0.tile([P, SLT], BF16, tag="bq")
            nc.gpsimd.partition_broadcast(bq[:], rrb[:])
            for k in range(kc):
                nc.vector.tensor_mul(lt[:, k, sl], lt[:, k, sl], bq[:])

    # per-head weight DMAs: emitted here so they queue behind aw on sync
    nc.sync.dma_start(kvb[:], d["kvb"].rearrange("(k p) m -> p k m", p=P))
    nc.sync.dma_start(qb[:], d["qb"].rearrange("(k p) m -> p k m", p=P))
    nc.sync.dma_start(ow[:], d["ow"])

    nrm0.release()
    x2p.release()
    ps_s.release()
    ps_a.release()
    ps_t.release()
    hidtp.release()
    hraw.release()
    awp.release()

    if dbg:
        nc.sync.dma_start(d["dbg_latq"], lat_q[:])
        nc.sync.dma_start(d["dbg_latkv"], lat_kv[:])
        nc.sync.dma_start(d["dbg_latpe"], lat_pe[:])

    if phase < 2:
        nc.sync.dma_start(d["out"][0:P, 0:T], lat_q[:, 0])
        latp.release()
        b1w.release()
        small.release()
        return

    # ============ qT / kT / V ============
    qkv = tc.alloc_tile_pool(name="qkv", bufs=1, side="right")
    qT = qkv.tile([P, HPC, T], BF16)
    kT = qkv.tile([P, HPC, T], BF16)
    V = qkv.tile([P, NT, HPC, D_V + 1], BF16)

    ps_k = tc.alloc_tile_pool(name="ps_k", bufs=2, space="PSUM")
    krop = tc.alloc_tile_pool(name="krop", bufs=1)

    # roped k_pe = [I32; I32]^T @ lat_pe[64:128]  (cross-partition add on PE)
    mad = krop.tile([P, D_ROPE], BF16)
    nc.sync.dma_start(mad[D_NOPE:P, :], d["madd"])
    kroT = krop.tile([D_ROPE, T], BF16)
    for c in range(4):
        kps = ps_k.tile([D_ROPE, SLT], F32, tag="krops")
        nc.tensor.matmul(kps[:], mad[D_NOPE:P, :],
                         lat_pe[D_NOPE:P, ts(c, SLT)],
                         start=True, stop=True)
        nc.vector.tensor_copy(kroT[:, ts(c, SLT)], kps[:])

    # kT nope rows from kv_b; V from kv_b v-cols
    kvr = kvb.rearrange("p k (h d) -> p k h d", h=HPC)
    for h in range(HPC):
        for c in range(NCH):
            kps = ps_k.tile([D_NOPE, CHW], F32, tag="kps")
            for half in range(2):
                hsl = ds(half * 512, 512)
                for k in range(KC_KV):
                    nc.tensor.matmul(
                        kps[:, hsl], kvr[:, k, h, 0:D_NOPE],
                        lat_kv[:, k, ds(c * CHW + half * 512, 512)],
                        start=(k == 0), stop=(k == KC_KV - 1),
                    )
            nc.vector.tensor_copy(kT[0:D_NOPE, h, ts(c, CHW)], kps[:])
        nc.sync.dma_start(kT[D_NOPE:D_NOPE + D_ROPE, h], kroT[:])
        nc.sync.dma_start(kT[D_NOPE + D_ROPE:P, h], kroT[:])
    krop.release()
    ps_k.release()

    ps_v = tc.alloc_tile_pool(name="ps_v", bufs=2, space="PSUM")
    vcols = kvr[:, :, :, D_NOPE:]
    for tt in range(NT):
        vps = ps_v.tile([P, HPC * D_V], F32, tag="vps")
        for k in range(KC_KV):
            nc.tensor.matmul(
                vps[:], lat_kv[:, k, ts(tt, P)], vcols[:, k],
                start=(k == 0), stop=(k == KC_KV - 1),
            )
        nc.vector.tensor_copy(
            V[:, tt, :, 1:D_V + 1], vps[:].rearrange("p (h d) -> p h d", h=HPC))
    nc.vector.tensor_copy(
        V[:, :, :, 0:1],
        ones_b[:, :, None, None].to_broadcast([P, NT, HPC, 1]))
    ps_v.release()

    # qT with fused rope multiplier on evacuation
    ps_q = tc.alloc_tile_pool(name="ps_q", bufs=3, space="PSUM")
    for c in range(NCH):
        for h in range(HPC):
            qps = ps_q.tile([P, CHW], F32, tag="qps")
            for half in range(2):
                hsl = ds(half * 512, 512)
                for k in range(KC_Q):
                    nc.tensor.matmul(
                        qps[:, hsl], qb[:, k, ts(h, P)],
                        lat_q[:, k, ds(c * CHW + half * 512, 512)],
                        start=(k == 0), stop=(k == KC_Q - 1),
                    )
            nc.vector.tensor_mul(qT[:, h, ts(c, CHW)], qps[:], rmf[:, ts(c, CHW)])
    ps_q.release()
    latp.release()

    if dbg:
        nc.sync.dma_start(d["dbg_qT"], qT[:])
        nc.sync.dma_start(d["dbg_kT"], kT[:])
        nc.sync.dma_start(d["dbg_V"], V[:])

    if phase < 3:
        nc.sync.dma_start(d["out"][0:P, 0:T], qT[:, 0])
        qkv.release()
        b1w.release()
        small.release()
        return

    # ============ attention + o_proj interleaved ============
    attnp = tc.alloc_tile_pool(name="attnp", bufs=1)
    attnT = attnp.tile([P, 3, T], BF16)
    nc.gpsimd.memset(attnT[D_NOPE:P, 2], 0.0)

    exp_p = tc.alloc_tile_pool(name="exp_p", bufs=4)
    nrm_p = tc.alloc_tile_pool(name="nrm_p", bufs=2)
    outp = tc.alloc_tile_pool(name="outp", bufs=2)
    ps_sc = tc.alloc_tile_pool(name="ps_sc", bufs=2, space="PSUM")
    ps_pv = tc.alloc_tile_pool(name="ps_pv", bufs=1, space="PSUM")
    ps_o = tc.alloc_tile_pool(name="ps_o", bufs=2, space="PSUM")

    def oproj_piece(c, piece, tail=False):
        """o_proj for t-tile `piece` (0..7) of attention chunk c.
        Interleaved pieces keep psum evacuation on DVE (ACT is exp-bound);
        tail pieces split DVE/ACT."""
        t = c * JPC + piece
        ob = outp.tile([P, HIDDEN], BF16, tag="ob")
        for n in range(HIDDEN // 512):
            ops = ps_o.tile([P, 512], F32, tag="ops")
            for kc in range(3):
                nc.tensor.matmul(
                    ops[:], attnT[:, kc, ts(t, P)], ow[:, kc, ts(n, 512)],
                    start=(kc == 0), stop=(kc == 2),
                )
            if tail and (t + n) % 2 == 1:
                nc.scalar.copy(ob[:, ts(n, 512)], ops[:])
            else:
                nc.vector.tensor_copy(ob[:, ts(n, 512)], ops[:])
        nc.sync.dma_start(d["out"][ts(t, P), :], ob[:])

    for c in range(NCH):
        for h in range(HPC):
            njt = JPC * (c + 1)
            pv = ps_pv.tile([D_V + 1, CHW], F32, tag="pv")
            for j in range(njt):
                d0 = max(0, P * (j - JPC * c))
                sps = ps_sc.tile([P, CHW], F32, tag="sps")
                for half in range(2):
                    lo = half * 512
                    hi = lo + 512
                    if hi <= d0:
                        continue
                    b0 = max(d0, lo)
                    nc.tensor.matmul(
                        sps[:, ds(b0, hi - b0)], kT[:, h, ts(j, P)],
                        qT[:, h, ds(c * CHW + b0, hi - b0)],
                        start=True, stop=True,
                    )
                ex = exp_p.tile([P, CHW], BF16, tag="ex")
                nc.scalar.activation(ex[:, d0:], sps[:, d0:], AF.Exp)
                if j >= JPC * c:
                    nc.vector.tensor_mul(
                        ex[:, ds(d0, P)], ex[:, ds(d0, P)], mask[:])
                # last j writing cols [0:512) is JPC*c+3 (d0 reaches 512 after)
                last_j = (JPC * c + 3, njt - 1)
                for half in range(2):
                    lo = half * 512
                    hi = lo + 512
                    if hi <= d0:
                        continue
                    b0 = max(d0, lo)
                    nc.tensor.matmul(
                        pv[:, ds(b0, hi - b0)], V[:, j, h], ex[:, ds(b0, hi - b0)],
                        start=(j == 0), stop=(j == last_j[half]),
                    )
            # evacuate pv fast, then normalize in SBUF.
            # denominator is pv row 0 (V ones column is first) — HW
            # partition_broadcast reads partition 0 only.
            pt = nrm_p.tile([D_V + 1, CHW], F32, tag="ptmp")
            nc.vector.tensor_copy(pt[:], pv[:])
            nc.vector.reciprocal(pt[0:1, :], pt[0:1, :])
            bcs = nrm_p.tile([D_V + 1, CHW], F32, tag="bcs")
            nc.gpsimd.partition_broadcast(bcs[:], pt[0:1, :])
            atmp = nrm_p.tile([D_V + 1, CHW], BF16, tag="atmp")
            nc.vector.tensor_mul(atmp[:], pt[:], bcs[:])
            nc.sync.dma_start(
                attnT[(h % 2) * D_V:(h % 2 + 1) * D_V, h // 2, ts(c, CHW)],
                atmp[1:, :])
            # interleave one o_proj piece of the previous chunk per head
            if phase >= 4 and c > 0 and h < HPC - 1:
                oproj_piece(c - 1, 2 * h)
                oproj_piece(c - 1, 2 * h + 1)
    if dbg:
        nc.sync.dma_start(d["dbg_attnT"], attnT[:])
    if phase < 4:
        nc.sync.dma_start(d["out"][0:P, 0:T], attnT[:, 0])
    else:
        # remaining o_proj: last chunk fully, and leftover of chunk NCH-2
        for piece in range(2 * (HPC - 1), JPC):
            oproj_piece(NCH - 2, piece, tail=True)
        for piece in range(JPC):
            oproj_piece(NCH - 1, piece, tail=True)

    ps_o.release()
    ps_pv.release()
    ps_sc.release()
    outp.release()
    nrm_p.release()
    exp_p.release()
    attnp.release()
    qkv.release()
    b1w.release()
    small.release()


def _build(dbg=False, repeat=1, phase=4):
    nc = bacc.Bacc("TRN2", target_bir_lowering=False, debug=False,
                   num_devices=NCORES)
    d = {
        "hid": nc.dram_tensor("hid", [T, HIDDEN], F32, kind="ExternalInput").ap(),
        "aw": nc.dram_tensor("aw", [HIDDEN, MTOT], BF16, kind="ExternalInput").ap(),
        "qb": nc.dram_tensor("qb", [Q_RANK, HPC * P], BF16, kind="ExternalInput").ap(),
        "kvb": nc.dram_tensor("kvb", [KV_RANK, HPC * P], BF16, kind="ExternalInput").ap(),
        "ow": nc.dram_tensor("ow", [P, 3, HIDDEN], BF16, kind="ExternalInput").ap(),
        "ropemul": nc.dram_tensor("ropemul", [P, T], F32, kind="ExternalInput").ap(),
        "madd": nc.dram_tensor("madd", [D_NOPE, D_ROPE], BF16, kind="ExternalInput").ap(),
        "out": nc.dram_tensor("out", [T, HIDDEN], BF16, kind="ExternalOutput").ap(),
    }
    if dbg:
        for nm, shp in (
            ("dbg_latq", [P, KC_Q, T]), ("dbg_latkv", [P, KC_KV, T]),
            ("dbg_latpe", [P, T]), ("dbg_qT", [P, HPC, T]),
            ("dbg_kT", [P, HPC, T]), ("dbg_V", [P, NT, HPC, D_V + 1]),
            ("dbg_attnT", [P, 3, T]),
        ):
            d[nm] = nc.dram_tensor(nm, shp, BF16, kind="ExternalOutput").ap()
    with tile.TileContext(nc) as tc:
        for _ in range(repeat):
            _body(nc, tc, d, dbg=dbg, phase=phase)
    nc.compile()
    return nc


def _bf16(x):
    import ml_dtypes
    return np.ascontiguousarray(np.asarray(x, np.float32).astype(ml_dtypes.bfloat16))


def _swap_neg(w):
    """Columns [-x2; x1] for neox rope, acting on the last axis of size 32."""
    return np.concatenate([-w[..., D2:], w[..., :D2]], axis=-1)


def make_in_maps(positions, hidden_states, q_a_w, q_a_ln, q_b_w, kv_a_w,
                 kv_a_ln, kv_b_w, o_w):
    pos = np.asarray(positions)
    hid = np.ascontiguousarray(np.asarray(hidden_states, dtype=np.float32))
    q_a_w = np.asarray(q_a_w, np.float32)
    q_a_ln = np.asarray(q_a_ln, np.float32)
    q_b_w = np.asarray(q_b_w, np.float32)
    kv_a_w = np.asarray(kv_a_w, np.float32)
    kv_a_ln = np.asarray(kv_a_ln, np.float32)
    kv_b_w = np.asarray(kv_b_w, np.float32)
    o_w = np.asarray(o_w, np.float32)

    # aw: [q | kv | pe | sw]
    pe_w = kv_a_w[:, KV_RANK:]                       # [HIDDEN, 32]
    aw = np.concatenate(
        [q_a_w, kv_a_w[:, :KV_RANK], pe_w, _swap_neg(pe_w)], axis=1)

    # qb: per head [nope64 | pe32 | sw32], ln & SCALE folded
    qb = (q_a_ln[:, None] * q_b_w * SCALE).reshape(Q_RANK, N_HEADS, D_QK)
    qb_ext = np.concatenate(
        [qb[:, :, :D_NOPE], qb[:, :, D_NOPE:], _swap_neg(qb[:, :, D_NOPE:])],
        axis=2)                                      # [Q_RANK, 40, 128]

    # kvb: per head [k_nope | v], ln folded
    kvb = (kv_a_ln[:, None] * kv_b_w).reshape(KV_RANK, N_HEADS, D_NOPE + D_V)

    # ropemul rows: 0:64 ones; 64:96 [c;c]; 96:128 [s;s]
    inv_freq = 1.0 / (ROPE_THETA ** (np.arange(0, D_ROPE, 2, np.float32) / D_ROPE))
    freqs = pos.astype(np.float32)[:, None] * inv_freq[None, :]   # [T, 16]
    cosv = np.cos(freqs).T                                        # [16, T]
    sinv = np.sin(freqs).T
    ropemul = np.concatenate(
        [np.ones((D_NOPE, T), np.float32), cosv, cosv, sinv, sinv], axis=0)

    in_maps = []
    for c in range(NCORES):
        h0 = c * HPC
        owc = o_w.reshape(N_HEADS, D_V, HIDDEN)[h0:h0 + HPC]      # [5, 64, H]
        ow3 = np.zeros((3, P, HIDDEN), np.float32)
        ow3[0] = owc[0:2].reshape(P, HIDDEN)
        ow3[1] = owc[2:4].reshape(P, HIDDEN)
        ow3[2, :D_V] = owc[4]
        in_maps.append({
            "hid": hid,
            "aw": _bf16(aw),
            "qb": _bf16(qb_ext[:, h0:h0 + HPC].reshape(Q_RANK, HPC * P)),
            "kvb": _bf16(kvb[:, h0:h0 + HPC].reshape(KV_RANK, HPC * P)),
            "ow": _bf16(ow3.transpose(1, 0, 2)),
            "ropemul": np.ascontiguousarray(ropemul),
            "madd": _bf16(np.concatenate([np.eye(D_ROPE, dtype=np.float32)] * 2, axis=0)),
        })
    return in_maps


def kernel(positions, hidden_states, q_a_w, q_a_ln, q_b_w, kv_a_w, kv_a_ln,
           kv_b_w, o_w, trace=False):
    global LAST_RESULT
    in_maps = make_in_maps(positions, hidden_states, q_a_w, q_a_ln, q_b_w,
                           kv_a_w, kv_a_ln, kv_b_w, o_w)
    nc = _build()
    res = run_bass_kernel_spmd(nc, in_maps, core_ids=list(range(NCORES)),
                               trace=trace)
    LAST_RESULT = res
    acc = np.zeros((T, HIDDEN), np.float64)
    for c in range(NCORES):
        acc += np.asarray(res.results[c]["out"], np.float64)
    return acc.astype(np.float32)


# revision 11
# speedup vs baseline: 7.5933x; 1.0865x over previous
"""MiniCPM3 attention block on 8 Trainium2 NeuronCores — v2.

Sharding: tensor-parallel over heads (5 heads/core); q_b/kv_b column-parallel,
o_proj row-parallel (host sums the 8 partial outputs). The low-rank
a-projection preamble is replicated per core (collectives have a ~78ms
per-call fixed cost in this environment and are avoided).

Key tricks vs v1:
- bf16 everywhere on-chip (weights pre-cast host-side); fp32 PSUM accums.
- RoPE via weights: the host appends "swapped" columns (sw = [-x2; x1]) to
  the pe slices of kv_a_w and q_b_w. Roping then becomes an elementwise
  multiply by [cos;cos]/[sin;sin] rows (fused into the PSUM evacuation for
  q) plus, for k only, a cross-partition add done with two identity
  matmuls. The q-side add happens implicitly inside the scores matmul by
  duplicating the roped k rows (contraction over 128 rows).
- ln and SCALE folded into q_b/kv_b host-side.
- attention chunks of 1024 columns (one exp per (h,j)), o_proj interleaved
  between attention heads so PE fills ACT-bound gaps.
- bf16 output, cast+summed on host.
"""

import numpy as np

import concourse.bass as bass
import concourse.tile as tile
import concourse.mybir as mybir
from concourse import bacc
from concourse.bass_utils import run_bass_kernel_spmd
from concourse.masks import make_identity

# ---- problem constants (hardcoded per spec) ----
T = 2048
HIDDEN = 2560
N_HEADS = 40
D_NOPE = 64
D_ROPE = 32
D_QK = 96
D_V = 64
Q_RANK = 768
KV_RANK = 256
ROPE_THETA = 10000.0
EPS = 1e-6
SCALE = float(D_QK) ** -0.5

NCORES = 8
HPC = N_HEADS // NCORES  # 5
P = 128
D2 = D_ROPE // 2  # 16

F32 = mybir.dt.float32
F32R = mybir.dt.float32r
BF16 = mybir.dt.bfloat16
AF = mybir.ActivationFunctionType
ALU = mybir.AluOpType

KC_HID = HIDDEN // P          # 20
KC_Q = Q_RANK // P            # 6
KC_KV = KV_RANK // P          # 2
MTOT = Q_RANK + KV_RANK + 2 * D_ROPE  # 1088 (q | kv | pe | sw)
NT = T // P                   # 16
NSL = 4                       # preamble token slices
SLT = T // NSL                # 512
CHW = 1024                    # attention tq chunk width
NCH = T // CHW                # 2
JPC = CHW // P                # 8 k-blocks per chunk

LAST_RESULT = None
ts = bass.ts
ds = bass.ds


def _body(nc, tc, d, dbg=False, phase=4):
    small = tc.alloc_tile_pool(name="small", bufs=1)
    ident = small.tile([P, P], F32)
    make_identity(nc, ident[:])
    # mask[i, j] = 1 if i <= j else 0 (tk row i <= tq col j in diag block)
    maskf = small.tile([P, P], F32)
    nc.gpsimd.memset(maskf[:], 1.0)
    nc.gpsimd.affine_select(
        out=maskf[:], in_=maskf[:], compare_op=ALU.is_ge,
        fill=0.0, base=0, pattern=[[1, P]], channel_multiplier=-1,
    )
    mask = small.tile([P, P], BF16)
    nc.vector.tensor_copy(mask[:], maskf[:])
    ones_b = small.tile([P, 1], BF16)
    nc.gpsimd.memset(ones_b[:], 1.0)
    eps_c = small.tile([1, 1], F32)
    nc.gpsimd.memset(eps_c[:], EPS)

    # rope multiplier rows: 0:64 ones, 64:96 [c;c], 96:128 [s;s]
    rmf = small.tile([P, T], F32)

    # per-head weights (live until o_proj); tiles allocated now, DMAs
    # emitted at the end of the preamble so aw wins the sync queue
    b1w = tc.alloc_tile_pool(name="b1w", bufs=1, side="right")
    qb = b1w.tile([P, KC_Q, HPC * P], BF16)
    kvb = b1w.tile([P, KC_KV, HPC * P], BF16)
    ow = b1w.tile([P, 3, HIDDEN], BF16)

    latp = tc.alloc_tile_pool(name="latp", bufs=1)
    lat_q = latp.tile([P, KC_Q, T], BF16)
    lat_kv = latp.tile([P, KC_KV, T], BF16)
    lat_pe = latp.tile([P, T], BF16)  # rows 64:128: pe*cos / sw*sin

    # ============ preamble: lat = a_proj(hidden^T) ============
    awp = tc.alloc_tile_pool(name="awp", bufs=1)
    aw = awp.tile([P, KC_HID, MTOT], BF16)
    nc.sync.dma_start(aw[:], d["aw"].rearrange("(k p) m -> p k m", p=P))
    nc.sync.dma_start(rmf[:], d["ropemul"])

    hraw = tc.alloc_tile_pool(name="hraw", bufs=2)
    hidtp = tc.alloc_tile_pool(name="hidtp", bufs=2)
    ps_t = tc.alloc_tile_pool(name="ps_t", bufs=3, space="PSUM")
    ps_a = tc.alloc_tile_pool(name="ps_a", bufs=3, space="PSUM")
    ps_s = tc.alloc_tile_pool(name="ps_s", bufs=1, space="PSUM")
    x2p = tc.alloc_tile_pool(name="x2p", bufs=3)
    nrm0 = tc.alloc_tile_pool(name="nrm0", bufs=2)

    # m-chunks: 6 q + 2 kv + 1 pe(64 rows at partition 64)
    mlist = [("q", i) for i in range(KC_Q)] + [("kv", i) for i in range(KC_KV)]
    mlist.append(("pe", 0))

    def emit_transposes(s):
        hidT = hidtp.tile([P, KC_HID, SLT], BF16, tag="hidT",
                          name="hidT%d" % s)
        for tt in range(SLT // P):
            hr = hraw.tile([P, HIDDEN], F32, tag="hr")
            nc.scalar.dma_start(hr[:], d["hid"][ts(s * (SLT // P) + tt, P), :])
            for k4 in range(KC_HID // 4):
                pt = ps_t.tile([P, 4, P], F32, tag="pt")
                for dk in range(4):
                    nc.tensor.transpose(pt[:, dk], hr[:, ts(4 * k4 + dk, P)],
                                        ident[:])
                nc.vector.tensor_copy(
                    hidT[:, 4 * k4:4 * k4 + 4, ts(tt, P)], pt[:])
        return hidT

    hidT_next = emit_transposes(0)
    for s in range(NSL):
        sl = ts(s, SLT)
        hidT = hidT_next
        if s + 1 < NSL:
            hidT_next = emit_transposes(s + 1)
        sqq = ps_s.tile([1, SLT], F32, tag="sqq")
        sqkv = ps_s.tile([1, SLT], F32, tag="sqkv")
        for kind, mi in mlist:
            pm = ps_a.tile([P, SLT], F32, tag="pm")
            if kind == "q":
                w_ap = aw[:, :, ts(mi, P)]
                pmv = pm[:]
            elif kind == "kv":
                w_ap = aw[:, :, ds(Q_RANK + mi * P, P)]
                pmv = pm[:]
            else:
                w_ap = aw[:, :, ds(Q_RANK + KV_RANK, 2 * D_ROPE)]
                pmv = pm[D_NOPE:P]
            for k in range(KC_HID):
                nc.tensor.matmul(
                    pmv, w_ap[:, k], hidT[:, k],
                    start=(k == 0), stop=(k == KC_HID - 1),
                )
            if kind == "pe":
                # fuse the rope multiplier into evacuation (no norm on pe)
                nc.vector.tensor_mul(
                    lat_pe[D_NOPE:P, sl], pmv, rmf[D_NOPE:P, sl])
            else:
                x2 = x2p.tile([P, SLT], BF16, tag="x2")
                nc.scalar.activation(x2[:], pm[:], AF.Square)
                sq = sqq if kind == "q" else sqkv
                kc = KC_Q if kind == "q" else KC_KV
                nc.tensor.matmul(
                    sq[:], ones_b[:], x2[:],
                    start=(mi == 0), stop=(mi == kc - 1),
                )
                dst = lat_q[:, mi, sl] if kind == "q" else lat_kv[:, mi, sl]
                nc.vector.tensor_copy(dst, pm[:])
        # rmsnorm scale for this token slice
        for name_, sq, rank, lt, kc in (
            ("q", sqq, Q_RANK, lat_q, KC_Q),
            ("kv", sqkv, KV_RANK, lat_kv, KC_KV),
        ):
            rr = nrm0.tile([1, SLT], F32, tag="rr")
            nc.scalar.activation(rr[:], sq[:], AF.Sqrt,
                                 bias=eps_c[:], scale=1.0 / rank)
            nc.vector.reciprocal(rr[:], rr[:])
            rrb = nrm0.tile([1, SLT], BF16, tag="rrb")
            nc.vector.tensor_copy(rrb[:], rr[:])
            bq = nrm0.tile([P, SLT], BF16, tag="bq")
            nc.gpsimd.partition_broadcast(bq[:], rrb[:])
            for k in range(kc):
                nc.vector.tensor_mul(lt[:, k, sl], lt[:, k, sl], bq[:])

    # per-head weight DMAs: emitted here so they queue behind aw on sync
    nc.sync.dma_start(kvb[:], d["kvb"].rearrange("(k p) m -> p k m", p=P))
    nc.sync.dma_start(qb[:], d["qb"].rearrange("(k p) m -> p k m", p=P))
    nc.sync.dma_start(ow[:], d["ow"])

    nrm0.release()
    x2p.release()
    ps_s.release()
    ps_a.release()
    ps_t.release()
    hidtp.release()
    hraw.release()
    awp.release()

    if dbg:
        nc.sync.dma_start(d["dbg_latq"], lat_q[:])
        nc.sync.dma_start(d["dbg_latkv"], lat_kv[:])
        nc.sync.dma_start(d["dbg_latpe"], lat_pe[:])

    if phase < 2:
        nc.sync.dma_start(d["out"][0:P, 0:T], lat_q[:, 0])
        latp.release()
        b1w.release()
        small.release()
        return

    # ============ qT / kT / V ============
    qkv = tc.alloc_tile_pool(name="qkv", bufs=1, side="right")
    qT = qkv.tile([P, HPC, T], BF16)
    kT = qkv.tile([P, HPC, T], BF16)
    V = qkv.tile([P, NT, HPC, D_V + 1], BF16)

    ps_k = tc.alloc_tile_pool(name="ps_k", bufs=2, space="PSUM")
    krop = tc.alloc_tile_pool(name="krop", bufs=1)

    # roped k_pe = [I32; I32]^T @ lat_pe[64:128]  (cross-partition add on PE)
    mad = krop.tile([P, D_ROPE], BF16)
    nc.sync.dma_start(mad[D_NOPE:P, :], d["madd"])
    kroT = krop.tile([D_ROPE, T], BF16)
    for c in range(4):
        kps = ps_k.tile([D_ROPE, SLT], F32, tag="krops")
        nc.tensor.matmul(kps[:], mad[D_NOPE:P, :],
                         lat_pe[D_NOPE:P, ts(c, SLT)],
                         start=True, stop=True)
        nc.vector.tensor_copy(kroT[:, ts(c, SLT)], kps[:])

    # kT nope rows from kv_b; V from kv_b v-cols
    kvr = kvb.rearrange("p k (h d) -> p k h d", h=HPC)
    for h in range(HPC):
        for c in range(NCH):
            kps = ps_k.tile([D_NOPE, CHW], F32, tag="kps")
            for half in range(2):
                hsl = ds(half * 512, 512)
                for k in range(KC_KV):
                    nc.tensor.matmul(
                        kps[:, hsl], kvr[:, k, h, 0:D_NOPE],
                        lat_kv[:, k, ds(c * CHW + half * 512, 512)],
                        start=(k == 0), stop=(k == KC_KV - 1),
                    )
            nc.scalar.copy(kT[0:D_NOPE, h, ts(c, CHW)], kps[:])
        nc.sync.dma_start(kT[D_NOPE:D_NOPE + D_ROPE, h], kroT[:])
        nc.sync.dma_start(kT[D_NOPE + D_ROPE:P, h], kroT[:])
    krop.release()
    ps_k.release()

    ps_v = tc.alloc_tile_pool(name="ps_v", bufs=2, space="PSUM")
    vcols = kvr[:, :, :, D_NOPE:]
    for tt in range(NT):
        vps = ps_v.tile([P, HPC * D_V], F32, tag="vps")
        for k in range(KC_KV):
            nc.tensor.matmul(
                vps[:], lat_kv[:, k, ts(tt, P)], vcols[:, k],
                start=(k == 0), stop=(k == KC_KV - 1),
            )
        nc.scalar.copy(
            V[:, tt, :, 1:D_V + 1], vps[:].rearrange("p (h d) -> p h d", h=HPC))
    nc.vector.tensor_copy(
        V[:, :, :, 0:1],
        ones_b[:, :, None, None].to_broadcast([P, NT, HPC, 1]))
    ps_v.release()

    # qT with fused rope multiplier on evacuation
    ps_q = tc.alloc_tile_pool(name="ps_q", bufs=3, space="PSUM")
    for c in range(NCH):
        for h in range(HPC):
            qps = ps_q.tile([P, CHW], F32, tag="qps")
            for half in range(2):
                hsl = ds(half * 512, 512)
                for k in range(KC_Q):
                    nc.tensor.matmul(
                        qps[:, hsl], qb[:, k, ts(h, P)],
                        lat_q[:, k, ds(c * CHW + half * 512, 512)],
                        start=(k == 0), stop=(k == KC_Q - 1),
                    )
            nc.vector.tensor_mul(qT[:, h, ts(c, CHW)], qps[:], rmf[:, ts(c, CHW)])
    ps_q.release()
    latp.release()

    if dbg:
        nc.sync.dma_start(d["dbg_qT"], qT[:])
        nc.sync.dma_start(d["dbg_kT"], kT[:])
        nc.sync.dma_start(d["dbg_V"], V[:])

    if phase < 3:
        nc.sync.dma_start(d["out"][0:P, 0:T], qT[:, 0])
        qkv.release()
        b1w.release()
        small.release()
        return

    # ============ attention + o_proj interleaved ============
    attnp = tc.alloc_tile_pool(name="attnp", bufs=1)
    attnT = attnp.tile([P, 3, T], BF16)
    nc.gpsimd.memset(attnT[D_NOPE:P, 2], 0.0)

    exp_p = tc.alloc_tile_pool(name="exp_p", bufs=4)
    nrm_p = tc.alloc_tile_pool(name="nrm_p", bufs=2)
    outp = tc.alloc_tile_pool(name="outp", bufs=2)
    ps_sc = tc.alloc_tile_pool(name="ps_sc", bufs=2, space="PSUM")
    ps_pv = tc.alloc_tile_pool(name="ps_pv", bufs=1, space="PSUM")
    ps_o = tc.alloc_tile_pool(name="ps_o", bufs=2, space="PSUM")

    def oproj_piece(c, piece, tail=False):
        """o_proj for t-tile `piece` (0..7) of attention chunk c.
        Interleaved pieces keep psum evacuation on DVE (ACT is exp-bound);
        tail pieces split DVE/ACT."""
        t = c * JPC + piece
        ob = outp.tile([P, HIDDEN], BF16, tag="ob")
        for n in range(HIDDEN // 512):
            ops = ps_o.tile([P, 512], F32, tag="ops")
            for kc in range(3):
                nc.tensor.matmul(
                    ops[:], attnT[:, kc, ts(t, P)], ow[:, kc, ts(n, 512)],
                    start=(kc == 0), stop=(kc == 2),
                )
            if tail and (t + n) % 2 == 1:
                nc.scalar.copy(ob[:, ts(n, 512)], ops[:])
            else:
                nc.vector.tensor_copy(ob[:, ts(n, 512)], ops[:])
        nc.sync.dma_start(d["out"][ts(t, P), :], ob[:])

    for c in range(NCH):
        for h in range(HPC):
            njt = JPC * (c + 1)
            pv = ps_pv.tile([D_V + 1, CHW], F32, tag="pv")
            for j in range(njt):
                d0 = max(0, P * (j - JPC * c))
                sps = ps_sc.tile([P, CHW], F32, tag="sps")
                for half in range(2):
                    lo = half * 512
                    hi = lo + 512
                    if hi <= d0:
                        continue
                    b0 = max(d0, lo)
                    nc.tensor.matmul(
                        sps[:, ds(b0, hi - b0)], kT[:, h, ts(j, P)],
                        qT[:, h, ds(c * CHW + b0, hi - b0)],
                        start=True, stop=True,
                    )
                ex = exp_p.tile([P, CHW], BF16, tag="ex")
                nc.scalar.activation(ex[:, d0:], sps[:, d0:], AF.Exp)
                if j >= JPC * c:
                    nc.vector.tensor_mul(
                        ex[:, ds(d0, P)], ex[:, ds(d0, P)], mask[:])
                # last j writing cols [0:512) is JPC*c+3 (d0 reaches 512 after)
                last_j = (JPC * c + 3, njt - 1)
                for half in range(2):
                    lo = half * 512
                    hi = lo + 512
                    if hi <= d0:
                        continue
                    b0 = max(d0, lo)
                    nc.tensor.matmul(
                        pv[:, ds(b0, hi - b0)], V[:, j, h], ex[:, ds(b0, hi - b0)],
                        start=(j == 0), stop=(j == last_j[half]),
                    )
            # evacuate pv fast, then normalize in SBUF.
            # denominator is pv row 0 (V ones column is first) — HW
            # partition_broadcast reads partition 0 only.
            pt = nrm_p.tile([D_V + 1, CHW], F32, tag="ptmp")
            nc.vector.tensor_copy(pt[:], pv[:])
            nc.vector.reciprocal(pt[0:1, :], pt[0:1, :])
            bcs = nrm_p.tile([D_V + 1, CHW], F32, tag="bcs")
            nc.gpsimd.partition_broadcast(bcs[:], pt[0:1, :])
            atmp = nrm_p.tile([D_V + 1, CHW], BF16, tag="atmp")
            nc.vector.tensor_mul(atmp[:], pt[:], bcs[:])
            nc.sync.dma_start(
                attnT[(h % 2) * D_V:(h % 2 + 1) * D_V, h // 2, ts(c, CHW)],
                atmp[1:, :])
            # interleave one o_proj piece of the previous chunk per head
            if phase >= 4 and c > 0 and h < HPC - 1:
                oproj_piece(c - 1, 2 * h)
                oproj_piece(c - 1, 2 * h + 1)
    if dbg:
        nc.sync.dma_start(d["dbg_attnT"], attnT[:])
    if phase < 4:
        nc.sync.dma_start(d["out"][0:P, 0:T], attnT[:, 0])
    else:
        # remaining o_proj: last chunk fully, and leftover of chunk NCH-2
        for piece in range(2 * (HPC - 1), JPC):
            oproj_piece(NCH - 2, piece, tail=True)
        for piece in range(JPC):
            oproj_piece(NCH - 1, piece, tail=True)

    ps_o.release()
    ps_pv.release()
    ps_sc.release()
    outp.release()
    nrm_p.release()
    exp_p.release()
    attnp.release()
    qkv.release()
    b1w.release()
    small.release()


def _build(dbg=False, repeat=1, phase=4):
    nc = bacc.Bacc("TRN2", target_bir_lowering=False, debug=False,
                   num_devices=NCORES)
    d = {
        "hid": nc.dram_tensor("hid", [T, HIDDEN], F32, kind="ExternalInput").ap(),
        "aw": nc.dram_tensor("aw", [HIDDEN, MTOT], BF16, kind="ExternalInput").ap(),
        "qb": nc.dram_tensor("qb", [Q_RANK, HPC * P], BF16, kind="ExternalInput").ap(),
        "kvb": nc.dram_tensor("kvb", [KV_RANK, HPC * P], BF16, kind="ExternalInput").ap(),
        "ow": nc.dram_tensor("ow", [P, 3, HIDDEN], BF16, kind="ExternalInput").ap(),
        "ropemul": nc.dram_tensor("ropemul", [P, T], F32, kind="ExternalInput").ap(),
        "madd": nc.dram_tensor("madd", [D_NOPE, D_ROPE], BF16, kind="ExternalInput").ap(),
        "out": nc.dram_tensor("out", [T, HIDDEN], BF16, kind="ExternalOutput").ap(),
    }
    if dbg:
        for nm, shp in (
            ("dbg_latq", [P, KC_Q, T]), ("dbg_latkv", [P, KC_KV, T]),
            ("dbg_latpe", [P, T]), ("dbg_qT", [P, HPC, T]),
            ("dbg_kT", [P, HPC, T]), ("dbg_V", [P, NT, HPC, D_V + 1]),
            ("dbg_attnT", [P, 3, T]),
        ):
            d[nm] = nc.dram_tensor(nm, shp, BF16, kind="ExternalOutput").ap()
    with tile.TileContext(nc) as tc:
        for _ in range(repeat):
            _body(nc, tc, d, dbg=dbg, phase=phase)
    nc.compile()
    return nc


def _bf16(x):
    import ml_dtypes
    return np.ascontiguousarray(np.asarray(x, np.float32).astype(ml_dtypes.bfloat16))


def _swap_neg(w):
    """Columns [-x2; x1] for neox rope, acting on the last axis of size 32."""
    return np.concatenate([-w[..., D2:], w[..., :D2]], axis=-1)


def make_in_maps(positions, hidden_states, q_a_w, q_a_ln, q_b_w, kv_a_w,
                 kv_a_ln, kv_b_w, o_w):
    pos = np.asarray(positions)
    hid = np.ascontiguousarray(np.asarray(hidden_states, dtype=np.float32))
    q_a_w = np.asarray(q_a_w, np.float32)
    q_a_ln = np.asarray(q_a_ln, np.float32)
    q_b_w = np.asarray(q_b_w, np.float32)
    kv_a_w = np.asarray(kv_a_w, np.float32)
    kv_a_ln = np.asarray(kv_a_ln, np.float32)
    kv_b_w = np.asarray(kv_b_w, np.float32)
    o_w = np.asarray(o_w, np.float32)

    # aw: [q | kv | pe | sw]
    pe_w = kv_a_w[:, KV_RANK:]                       # [HIDDEN, 32]
    aw = np.concatenate(
        [q_a_w, kv_a_w[:, :KV_RANK], pe_w, _swap_neg(pe_w)], axis=1)

    # qb: per head [nope64 | pe32 | sw32], ln & SCALE folded
    qb = (q_a_ln[:, None] * q_b_w * SCALE).reshape(Q_RANK, N_HEADS, D_QK)
    qb_ext = np.concatenate(
        [qb[:, :, :D_NOPE], qb[:, :, D_NOPE:], _swap_neg(qb[:, :, D_NOPE:])],
        axis=2)                                      # [Q_RANK, 40, 128]

    # kvb: per head [k_nope | v], ln folded
    kvb = (kv_a_ln[:, None] * kv_b_w).reshape(KV_RANK, N_HEADS, D_NOPE + D_V)

    # ropemul rows: 0:64 ones; 64:96 [c;c]; 96:128 [s;s]
    inv_freq = 1.0 / (ROPE_THETA ** (np.arange(0, D_ROPE, 2, np.float32) / D_ROPE))
    freqs = pos.astype(np.float32)[:, None] * inv_freq[None, :]   # [T, 16]
    cosv = np.cos(freqs).T                                        # [16, T]
    sinv = np.sin(freqs).T
    ropemul = np.concatenate(
        [np.ones((D_NOPE, T), np.float32), cosv, cosv, sinv, sinv], axis=0)

    in_maps = []
    for c in range(NCORES):
        h0 = c * HPC
        owc = o_w.reshape(N_HEADS, D_V, HIDDEN)[h0:h0 + HPC]      # [5, 64, H]
        ow3 = np.zeros((3, P, HIDDEN), np.float32)
        ow3[0] = owc[0:2].reshape(P, HIDDEN)
        ow3[1] = owc[2:4].reshape(P, HIDDEN)
        ow3[2, :D_V] = owc[4]
        in_maps.append({
            "hid": hid,
            "aw": _bf16(aw),
            "qb": _bf16(qb_ext[:, h0:h0 + HPC].reshape(Q_RANK, HPC * P)),
            "kvb": _bf16(kvb[:, h0:h0 + HPC].reshape(KV_RANK, HPC * P)),
            "ow": _bf16(ow3.transpose(1, 0, 2)),
            "ropemul": np.ascontiguousarray(ropemul),
            "madd": _bf16(np.concatenate([np.eye(D_ROPE, dtype=np.float32)] * 2, axis=0)),
        })
    return in_maps


def kernel(positions, hidden_states, q_a_w, q_a_ln, q_b_w, kv_a_w, kv_a_ln,
           kv_b_w, o_w, trace=False):
    global LAST_RESULT
    in_maps = make_in_maps(positions, hidden_states, q_a_w, q_a_ln, q_b_w,
                           kv_a_w, kv_a_ln, kv_b_w, o_w)
    nc = _build()
    res = run_bass_kernel_spmd(nc, in_maps, core_ids=list(range(NCORES)),
                               trace=trace)
    LAST_RESULT = res
    acc = np.zeros((T, HIDDEN), np.float64)
    for c in range(NCORES):
        acc += np.asarray(res.results[c]["out"], np.float64)
    return acc.astype(np.float32)


# revision 12
# speedup vs baseline: 8.5493x; 1.1259x over previous
"""MiniCPM3 attention block on 8 Trainium2 NeuronCores — v2.

Sharding: tensor-parallel over heads (5 heads/core); q_b/kv_b column-parallel,
o_proj row-parallel (host sums the 8 partial outputs). The low-rank
a-projection preamble is replicated per core (collectives have a ~78ms
per-call fixed cost in this environment and are avoided).

Key tricks vs v1:
- bf16 everywhere on-chip (weights pre-cast host-side); fp32 PSUM accums.
- RoPE via weights: the host appends "swapped" columns (sw = [-x2; x1]) to
  the pe slices of kv_a_w and q_b_w. Roping then becomes an elementwise
  multiply by [cos;cos]/[sin;sin] rows (fused into the PSUM evacuation for
  q) plus, for k only, a cross-partition add done with two identity
  matmuls. The q-side add happens implicitly inside the scores matmul by
  duplicating the roped k rows (contraction over 128 rows).
- ln and SCALE folded into q_b/kv_b host-side.
- attention chunks of 1024 columns (one exp per (h,j)), o_proj interleaved
  between attention heads so PE fills ACT-bound gaps.
- bf16 output, cast+summed on host.
"""

import numpy as np

import concourse.bass as bass
import concourse.tile as tile
import concourse.mybir as mybir
from concourse import bacc
from concourse.bass_utils import run_bass_kernel_spmd
from concourse.masks import make_identity

# ---- problem constants (hardcoded per spec) ----
T = 2048
HIDDEN = 2560
N_HEADS = 40
D_NOPE = 64
D_ROPE = 32
D_QK = 96
D_V = 64
Q_RANK = 768
KV_RANK = 256
ROPE_THETA = 10000.0
EPS = 1e-6
SCALE = float(D_QK) ** -0.5

NCORES = 8
HPC = N_HEADS // NCORES  # 5
P = 128
D2 = D_ROPE // 2  # 16

F32 = mybir.dt.float32
F32R = mybir.dt.float32r
BF16 = mybir.dt.bfloat16
AF = mybir.ActivationFunctionType
ALU = mybir.AluOpType

KC_HID = HIDDEN // P          # 20
KC_Q = Q_RANK // P            # 6
KC_KV = KV_RANK // P          # 2
MTOT = Q_RANK + KV_RANK + 2 * D_ROPE  # 1088 (q | kv | pe | sw)
NT = T // P                   # 16
NSL = 4                       # preamble token slices
SLT = T // NSL                # 512
CHW = 1024                    # attention tq chunk width
NCH = T // CHW                # 2
JPC = CHW // P                # 8 k-blocks per chunk

LAST_RESULT = None
ts = bass.ts
ds = bass.ds


def _body(nc, tc, d, dbg=False, phase=4):
    small = tc.alloc_tile_pool(name="small", bufs=1)
    ident = small.tile([P, P], F32)
    make_identity(nc, ident[:])
    # mask[i, j] = 1 if i <= j else 0 (tk row i <= tq col j in diag block)
    maskf = small.tile([P, P], F32)
    nc.gpsimd.memset(maskf[:], 1.0)
    nc.gpsimd.affine_select(
        out=maskf[:], in_=maskf[:], compare_op=ALU.is_ge,
        fill=0.0, base=0, pattern=[[1, P]], channel_multiplier=-1,
    )
    mask = small.tile([P, P], BF16)
    nc.vector.tensor_copy(mask[:], maskf[:])
    ones_b = small.tile([P, 1], BF16)
    nc.gpsimd.memset(ones_b[:], 1.0)
    eps_c = small.tile([1, 1], F32)
    nc.gpsimd.memset(eps_c[:], EPS)

    # rope multiplier rows: 0:64 ones, 64:96 [c;c], 96:128 [s;s]
    rmf = small.tile([P, T], F32)

    # per-head weights (live until o_proj); tiles allocated now, DMAs
    # emitted at the end of the preamble so aw wins the sync queue
    b1w = tc.alloc_tile_pool(name="b1w", bufs=1, side="right")
    qb = b1w.tile([P, KC_Q, HPC * P], BF16)
    kvb = b1w.tile([P, KC_KV, HPC * P], BF16)
    ow = b1w.tile([P, 3, HIDDEN], BF16)

    latp = tc.alloc_tile_pool(name="latp", bufs=1)
    lat_q = latp.tile([P, KC_Q, T], BF16)
    lat_kv = latp.tile([P, KC_KV, T], BF16)
    lat_pe = latp.tile([P, T], BF16)  # rows 64:128: pe*cos / sw*sin

    # ============ preamble: lat = a_proj(hidden^T) ============
    awp = tc.alloc_tile_pool(name="awp", bufs=1)
    aw = awp.tile([P, KC_HID, MTOT], BF16)
    nc.sync.dma_start(aw[:], d["aw"].rearrange("(k p) m -> p k m", p=P))
    nc.sync.dma_start(rmf[:], d["ropemul"])

    hraw = tc.alloc_tile_pool(name="hraw", bufs=2)
    hidtp = tc.alloc_tile_pool(name="hidtp", bufs=2)
    ps_t = tc.alloc_tile_pool(name="ps_t", bufs=3, space="PSUM")
    ps_a = tc.alloc_tile_pool(name="ps_a", bufs=3, space="PSUM")
    ps_s = tc.alloc_tile_pool(name="ps_s", bufs=1, space="PSUM")
    x2p = tc.alloc_tile_pool(name="x2p", bufs=3)
    nrm0 = tc.alloc_tile_pool(name="nrm0", bufs=2)

    # m-chunks: 6 q + 2 kv + 1 pe(64 rows at partition 64)
    mlist = [("q", i) for i in range(KC_Q)] + [("kv", i) for i in range(KC_KV)]
    mlist.append(("pe", 0))

    def emit_transposes(s):
        hidT = hidtp.tile([P, KC_HID, SLT], BF16, tag="hidT",
                          name="hidT%d" % s)
        for tt in range(SLT // P):
            hr = hraw.tile([P, HIDDEN], F32, tag="hr")
            nc.scalar.dma_start(hr[:], d["hid"][ts(s * (SLT // P) + tt, P), :])
            for k4 in range(KC_HID // 4):
                pt = ps_t.tile([P, 4, P], F32, tag="pt")
                for dk in range(4):
                    nc.tensor.transpose(pt[:, dk], hr[:, ts(4 * k4 + dk, P)],
                                        ident[:])
                nc.vector.tensor_copy(
                    hidT[:, 4 * k4:4 * k4 + 4, ts(tt, P)], pt[:])
        return hidT

    hidT_next = emit_transposes(0)
    for s in range(NSL):
        sl = ts(s, SLT)
        hidT = hidT_next
        if s + 1 < NSL:
            hidT_next = emit_transposes(s + 1)
        sqq = ps_s.tile([1, SLT], F32, tag="sqq")
        sqkv = ps_s.tile([1, SLT], F32, tag="sqkv")
        for kind, mi in mlist:
            pm = ps_a.tile([P, SLT], F32, tag="pm")
            if kind == "q":
                w_ap = aw[:, :, ts(mi, P)]
                pmv = pm[:]
            elif kind == "kv":
                w_ap = aw[:, :, ds(Q_RANK + mi * P, P)]
                pmv = pm[:]
            else:
                w_ap = aw[:, :, ds(Q_RANK + KV_RANK, 2 * D_ROPE)]
                pmv = pm[D_NOPE:P]
            for k in range(KC_HID):
                nc.tensor.matmul(
                    pmv, w_ap[:, k], hidT[:, k],
                    start=(k == 0), stop=(k == KC_HID - 1),
                )
            if kind == "pe":
                # fuse the rope multiplier into evacuation (no norm on pe)
                nc.vector.tensor_mul(
                    lat_pe[D_NOPE:P, sl], pmv, rmf[D_NOPE:P, sl])
            else:
                x2 = x2p.tile([P, SLT], BF16, tag="x2")
                nc.scalar.activation(x2[:], pm[:], AF.Square)
                sq = sqq if kind == "q" else sqkv
                kc = KC_Q if kind == "q" else KC_KV
                nc.tensor.matmul(
                    sq[:], ones_b[:], x2[:],
                    start=(mi == 0), stop=(mi == kc - 1),
                )
                dst = lat_q[:, mi, sl] if kind == "q" else lat_kv[:, mi, sl]
                nc.vector.tensor_copy(dst, pm[:])
        # rmsnorm scale for this token slice
        for name_, sq, rank, lt, kc in (
            ("q", sqq, Q_RANK, lat_q, KC_Q),
            ("kv", sqkv, KV_RANK, lat_kv, KC_KV),
        ):
            rr = nrm0.tile([1, SLT], F32, tag="rr")
            nc.scalar.activation(rr[:], sq[:], AF.Sqrt,
                                 bias=eps_c[:], scale=1.0 / rank)
            nc.vector.reciprocal(rr[:], rr[:])
            rrb = nrm0.tile([1, SLT], BF16, tag="rrb")
            nc.vector.tensor_copy(rrb[:], rr[:])
            bq = nrm0.tile([P, SLT], BF16, tag="bq")
            nc.gpsimd.partition_broadcast(bq[:], rrb[:])
            for k in range(kc):
                nc.vector.tensor_mul(lt[:, k, sl], lt[:, k, sl], bq[:])

    # per-head weight DMAs: emitted here so they queue behind aw on sync
    nc.sync.dma_start(kvb[:], d["kvb"].rearrange("(k p) m -> p k m", p=P))
    nc.sync.dma_start(qb[:], d["qb"].rearrange("(k p) m -> p k m", p=P))
    nc.sync.dma_start(ow[:], d["ow"])

    nrm0.release()
    x2p.release()
    ps_s.release()
    ps_a.release()
    ps_t.release()
    hidtp.release()
    hraw.release()
    awp.release()

    if dbg:
        nc.sync.dma_start(d["dbg_latq"], lat_q[:])
        nc.sync.dma_start(d["dbg_latkv"], lat_kv[:])
        nc.sync.dma_start(d["dbg_latpe"], lat_pe[:])

    if phase < 2:
        nc.sync.dma_start(d["out"][0:P, 0:T], lat_q[:, 0])
        latp.release()
        b1w.release()
        small.release()
        return

    # ============ qT / kT / V ============
    qkv = tc.alloc_tile_pool(name="qkv", bufs=1, side="right")
    qT = qkv.tile([P, HPC, T], BF16)
    kT = qkv.tile([P, HPC, T], BF16)
    V = qkv.tile([P, NT, HPC, D_V + 1], BF16)

    ps_k = tc.alloc_tile_pool(name="ps_k", bufs=2, space="PSUM")
    krop = tc.alloc_tile_pool(name="krop", bufs=1)

    # roped k_pe = [I32; I32]^T @ lat_pe[64:128]  (cross-partition add on PE)
    mad = krop.tile([P, D_ROPE], BF16)
    nc.sync.dma_start(mad[D_NOPE:P, :], d["madd"])
    kroT = krop.tile([D_ROPE, T], BF16)
    for c in range(4):
        kps = ps_k.tile([D_ROPE, SLT], F32, tag="krops")
        nc.tensor.matmul(kps[:], mad[D_NOPE:P, :],
                         lat_pe[D_NOPE:P, ts(c, SLT)],
                         start=True, stop=True)
        nc.vector.tensor_copy(kroT[:, ts(c, SLT)], kps[:])

    # kT nope rows from kv_b; V from kv_b v-cols
    kvr = kvb.rearrange("p k (h d) -> p k h d", h=HPC)
    for h in range(HPC):
        for c in range(NCH):
            kps = ps_k.tile([D_NOPE, CHW], F32, tag="kps")
            for half in range(2):
                hsl = ds(half * 512, 512)
                for k in range(KC_KV):
                    nc.tensor.matmul(
                        kps[:, hsl], kvr[:, k, h, 0:D_NOPE],
                        lat_kv[:, k, ds(c * CHW + half * 512, 512)],
                        start=(k == 0), stop=(k == KC_KV - 1),
                    )
            nc.scalar.copy(kT[0:D_NOPE, h, ts(c, CHW)], kps[:])
        nc.sync.dma_start(kT[D_NOPE:D_NOPE + D_ROPE, h], kroT[:])
        nc.sync.dma_start(kT[D_NOPE + D_ROPE:P, h], kroT[:])
    krop.release()
    ps_k.release()

    ps_v = tc.alloc_tile_pool(name="ps_v", bufs=2, space="PSUM")
    vcols = kvr[:, :, :, D_NOPE:]
    for tt in range(NT):
        vps = ps_v.tile([P, HPC * D_V], F32, tag="vps")
        for k in range(KC_KV):
            nc.tensor.matmul(
                vps[:], lat_kv[:, k, ts(tt, P)], vcols[:, k],
                start=(k == 0), stop=(k == KC_KV - 1),
            )
        nc.scalar.copy(
            V[:, tt, :, 1:D_V + 1], vps[:].rearrange("p (h d) -> p h d", h=HPC))
    nc.vector.tensor_copy(
        V[:, :, :, 0:1],
        ones_b[:, :, None, None].to_broadcast([P, NT, HPC, 1]))
    ps_v.release()

    # qT with fused rope multiplier on evacuation
    ps_q = tc.alloc_tile_pool(name="ps_q", bufs=3, space="PSUM")
    for c in range(NCH):
        for h in range(HPC):
            qps = ps_q.tile([P, CHW], F32, tag="qps")
            for half in range(2):
                hsl = ds(half * 512, 512)
                for k in range(KC_Q):
                    nc.tensor.matmul(
                        qps[:, hsl], qb[:, k, ts(h, P)],
                        lat_q[:, k, ds(c * CHW + half * 512, 512)],
                        start=(k == 0), stop=(k == KC_Q - 1),
                    )
            nc.vector.tensor_mul(qT[:, h, ts(c, CHW)], qps[:], rmf[:, ts(c, CHW)])
    ps_q.release()
    latp.release()

    if dbg:
        nc.sync.dma_start(d["dbg_qT"], qT[:])
        nc.sync.dma_start(d["dbg_kT"], kT[:])
        nc.sync.dma_start(d["dbg_V"], V[:])

    if phase < 3:
        nc.sync.dma_start(d["out"][0:P, 0:T], qT[:, 0])
        qkv.release()
        b1w.release()
        small.release()
        return

    # ============ attention + o_proj interleaved ============
    attnp = tc.alloc_tile_pool(name="attnp", bufs=1)
    attnT = attnp.tile([P, 3, T], BF16)
    nc.gpsimd.memset(attnT[D_NOPE:P, 2], 0.0)

    exp_p = tc.alloc_tile_pool(name="exp_p", bufs=6)
    nrm_p = tc.alloc_tile_pool(name="nrm_p", bufs=3)
    outp = tc.alloc_tile_pool(name="outp", bufs=3)
    ps_sc = tc.alloc_tile_pool(name="ps_sc", bufs=2, space="PSUM")
    ps_pv = tc.alloc_tile_pool(name="ps_pv", bufs=1, space="PSUM")
    ps_o = tc.alloc_tile_pool(name="ps_o", bufs=2, space="PSUM")

    def oproj_piece(c, piece, tail=False):
        """o_proj for t-tile `piece` (0..7) of attention chunk c.
        Interleaved pieces keep psum evacuation on DVE (ACT is exp-bound);
        tail pieces split DVE/ACT."""
        t = c * JPC + piece
        ob = outp.tile([P, HIDDEN], BF16, tag="ob")
        for n in range(HIDDEN // 512):
            ops = ps_o.tile([P, 512], F32, tag="ops")
            for kc in range(3):
                nc.tensor.matmul(
                    ops[:], attnT[:, kc, ts(t, P)], ow[:, kc, ts(n, 512)],
                    start=(kc == 0), stop=(kc == 2),
                )
            if tail and (t + n) % 2 == 1:
                nc.scalar.copy(ob[:, ts(n, 512)], ops[:])
            else:
                nc.vector.tensor_copy(ob[:, ts(n, 512)], ops[:])
        nc.sync.dma_start(d["out"][ts(t, P), :], ob[:])

    for c in range(NCH):
        for h in range(HPC):
            njt = JPC * (c + 1)
            pv = ps_pv.tile([D_V + 1, CHW], F32, tag="pv")
            for j in range(njt):
                d0 = max(0, P * (j - JPC * c))
                sps = ps_sc.tile([P, CHW], F32, tag="sps")
                for half in range(2):
                    lo = half * 512
                    hi = lo + 512
                    if hi <= d0:
                        continue
                    b0 = max(d0, lo)
                    nc.tensor.matmul(
                        sps[:, ds(b0, hi - b0)], kT[:, h, ts(j, P)],
                        qT[:, h, ds(c * CHW + b0, hi - b0)],
                        start=True, stop=True,
                    )
                ex = exp_p.tile([P, CHW], BF16, tag="ex")
                nc.scalar.activation(ex[:, d0:], sps[:, d0:], AF.Exp)
                if j >= JPC * c:
                    nc.vector.tensor_mul(
                        ex[:, ds(d0, P)], ex[:, ds(d0, P)], mask[:])
                # last j writing cols [0:512) is JPC*c+3 (d0 reaches 512 after)
                last_j = (JPC * c + 3, njt - 1)
                for half in range(2):
                    lo = half * 512
                    hi = lo + 512
                    if hi <= d0:
                        continue
                    b0 = max(d0, lo)
                    nc.tensor.matmul(
                        pv[:, ds(b0, hi - b0)], V[:, j, h], ex[:, ds(b0, hi - b0)],
                        start=(j == 0), stop=(j == last_j[half]),
                    )
            # evacuate pv fast, then normalize in SBUF.
            # denominator is pv row 0 (V ones column is first) — HW
            # partition_broadcast reads partition 0 only.
            pt = nrm_p.tile([D_V + 1, CHW], F32, tag="ptmp")
            nc.vector.tensor_copy(pt[:], pv[:])
            nc.vector.reciprocal(pt[0:1, :], pt[0:1, :])
            bcs = nrm_p.tile([D_V + 1, CHW], F32, tag="bcs")
            nc.gpsimd.partition_broadcast(bcs[:], pt[0:1, :])
            atmp = nrm_p.tile([D_V + 1, CHW], BF16, tag="atmp")
            nc.vector.tensor_mul(atmp[:], pt[:], bcs[:])
            nc.sync.dma_start(
                attnT[(h % 2) * D_V:(h % 2 + 1) * D_V, h // 2, ts(c, CHW)],
                atmp[1:, :])
            # interleave one o_proj piece of the previous chunk per head
            if phase >= 4 and c > 0 and h < HPC - 1:
                oproj_piece(c - 1, 2 * h)
                oproj_piece(c - 1, 2 * h + 1)
    if dbg:
        nc.sync.dma_start(d["dbg_attnT"], attnT[:])
    if phase < 4:
        nc.sync.dma_start(d["out"][0:P, 0:T], attnT[:, 0])
    else:
        # remaining o_proj: last chunk fully, and leftover of chunk NCH-2
        for piece in range(2 * (HPC - 1), JPC):
            oproj_piece(NCH - 2, piece, tail=True)
        for piece in range(JPC):
            oproj_piece(NCH - 1, piece, tail=True)

    ps_o.release()
    ps_pv.release()
    ps_sc.release()
    outp.release()
    nrm_p.release()
    exp_p.release()
    attnp.release()
    qkv.release()
    b1w.release()
    small.release()


def _build(dbg=False, repeat=1, phase=4):
    nc = bacc.Bacc("TRN2", target_bir_lowering=False, debug=False,
                   num_devices=NCORES)
    d = {
        "hid": nc.dram_tensor("hid", [T, HIDDEN], F32, kind="ExternalInput").ap(),
        "aw": nc.dram_tensor("aw", [HIDDEN, MTOT], BF16, kind="ExternalInput").ap(),
        "qb": nc.dram_tensor("qb", [Q_RANK, HPC * P], BF16, kind="ExternalInput").ap(),
        "kvb": nc.dram_tensor("kvb", [KV_RANK, HPC * P], BF16, kind="ExternalInput").ap(),
        "ow": nc.dram_tensor("ow", [P, 3, HIDDEN], BF16, kind="ExternalInput").ap(),
        "ropemul": nc.dram_tensor("ropemul", [P, T], F32, kind="ExternalInput").ap(),
        "madd": nc.dram_tensor("madd", [D_NOPE, D_ROPE], BF16, kind="ExternalInput").ap(),
        "out": nc.dram_tensor("out", [T, HIDDEN], BF16, kind="ExternalOutput").ap(),
    }
    if dbg:
        for nm, shp in (
            ("dbg_latq", [P, KC_Q, T]), ("dbg_latkv", [P, KC_KV, T]),
            ("dbg_latpe", [P, T]), ("dbg_qT", [P, HPC, T]),
            ("dbg_kT", [P, HPC, T]), ("dbg_V", [P, NT, HPC, D_V + 1]),
            ("dbg_attnT", [P, 3, T]),
        ):
            d[nm] = nc.dram_tensor(nm, shp, BF16, kind="ExternalOutput").ap()
    with tile.TileContext(nc) as tc:
        for _ in range(repeat):
            _body(nc, tc, d, dbg=dbg, phase=phase)
    nc.compile()
    return nc


def _bf16(x):
    import ml_dtypes
    return np.ascontiguousarray(np.asarray(x, np.float32).astype(ml_dtypes.bfloat16))


def _swap_neg(w):
    """Columns [-x2; x1] for neox rope, acting on the last axis of size 32."""
    return np.concatenate([-w[..., D2:], w[..., :D2]], axis=-1)


def make_in_maps(positions, hidden_states, q_a_w, q_a_ln, q_b_w, kv_a_w,
                 kv_a_ln, kv_b_w, o_w):
    pos = np.asarray(positions)
    hid = np.ascontiguousarray(np.asarray(hidden_states, dtype=np.float32))
    q_a_w = np.asarray(q_a_w, np.float32)
    q_a_ln = np.asarray(q_a_ln, np.float32)
    q_b_w = np.asarray(q_b_w, np.float32)
    kv_a_w = np.asarray(kv_a_w, np.float32)
    kv_a_ln = np.asarray(kv_a_ln, np.float32)
    kv_b_w = np.asarray(kv_b_w, np.float32)
    o_w = np.asarray(o_w, np.float32)

    # aw: [q | kv | pe | sw]
    pe_w = kv_a_w[:, KV_RANK:]                       # [HIDDEN, 32]
    aw = np.concatenate(
        [q_a_w, kv_a_w[:, :KV_RANK], pe_w, _swap_neg(pe_w)], axis=1)

    # qb: per head [nope64 | pe32 | sw32], ln & SCALE folded
    qb = (q_a_ln[:, None] * q_b_w * SCALE).reshape(Q_RANK, N_HEADS, D_QK)
    qb_ext = np.concatenate(
        [qb[:, :, :D_NOPE], qb[:, :, D_NOPE:], _swap_neg(qb[:, :, D_NOPE:])],
        axis=2)                                      # [Q_RANK, 40, 128]

    # kvb: per head [k_nope | v], ln folded
    kvb = (kv_a_ln[:, None] * kv_b_w).reshape(KV_RANK, N_HEADS, D_NOPE + D_V)

    # ropemul rows: 0:64 ones; 64:96 [c;c]; 96:128 [s;s]
    inv_freq = 1.0 / (ROPE_THETA ** (np.arange(0, D_ROPE, 2, np.float32) / D_ROPE))
    freqs = pos.astype(np.float32)[:, None] * inv_freq[None, :]   # [T, 16]
    cosv = np.cos(freqs).T                                        # [16, T]
    sinv = np.sin(freqs).T
    ropemul = np.concatenate(
        [np.ones((D_NOPE, T), np.float32), cosv, cosv, sinv, sinv], axis=0)

    in_maps = []
    for c in range(NCORES):
        h0 = c * HPC
        owc = o_w.reshape(N_HEADS, D_V, HIDDEN)[h0:h0 + HPC]      # [5, 64, H]
        ow3 = np.zeros((3, P, HIDDEN), np.float32)
        ow3[0] = owc[0:2].reshape(P, HIDDEN)
        ow3[1] = owc[2:4].reshape(P, HIDDEN)
        ow3[2, :D_V] = owc[4]
        in_maps.append({
            "hid": hid,
            "aw": _bf16(aw),
            "qb": _bf16(qb_ext[:, h0:h0 + HPC].reshape(Q_RANK, HPC * P)),
            "kvb": _bf16(kvb[:, h0:h0 + HPC].reshape(KV_RANK, HPC * P)),
            "ow": _bf16(ow3.transpose(1, 0, 2)),
            "ropemul": np.ascontiguousarray(ropemul),
            "madd": _bf16(np.concatenate([np.eye(D_ROPE, dtype=np.float32)] * 2, axis=0)),
        })
    return in_maps


def kernel(positions, hidden_states, q_a_w, q_a_ln, q_b_w, kv_a_w, kv_a_ln,
           kv_b_w, o_w, trace=False):
    global LAST_RESULT
    in_maps = make_in_maps(positions, hidden_states, q_a_w, q_a_ln, q_b_w,
                           kv_a_w, kv_a_ln, kv_b_w, o_w)
    nc = _build()
    res = run_bass_kernel_spmd(nc, in_maps, core_ids=list(range(NCORES)),
                               trace=trace)
    LAST_RESULT = res
    acc = np.zeros((T, HIDDEN), np.float64)
    for c in range(NCORES):
        acc += np.asarray(res.results[c]["out"], np.float64)
    return acc.astype(np.float32)
